# revision 19
# baseline (speedup 1.0000x reference)
"""NeuronMemory retrieval kernel for 8 TRN2 NeuronCores — v4.

Per token: softmax-routed low-rank projection Q (rank 128), dense scores
against 32768 knowledge keys, top-8, softmax, weighted gather of V rows.
Sharding: data-parallel over the 4096 tokens (512/core); tables replicated.

Architecture (per core, 4 token tiles of 128):
  A. router scores (fp16 2-term split, prescaled) + softmax -> wts
  B. Q' = 128*Q via fp16 3-term-split matmuls + fused weighting (exact to
     ~2^-22); Wg streamed per tile so tile t+1's B overlaps tile t's C.
  T. PE-transpose -> QT (true scale SCALE*Q) -> QTh = fp16(QT) for screening
  C. screening scores s ~= QTh.T @ Kh (fp16 1-term, resident Kh) -> PSUM
     per 1024-chunk; Scalar copies +768 (rounds mantissa to 2^-14 grid);
     Vector packs p = (s768-768) + idx*2^-26 (chunk-local index embedded in
     low mantissa bits); Vector max8 per 2048 super-chunk -> 8 packed
     candidates each (16 super-chunks -> 128 candidates, provably contains
     the true top-8 up to fp16 screen noise ~3e-4 rel, covered by margin).
  D. merge: max8 + match_replace + max8 -> top-12 packed finalists; unpack
     value+index; resolve super-chunk base via is_equal one-hot dot with a
     per-slot base table.
  E. exact rescore: indirect-DMA gather the 12 candidate K rows (fp32) and
     dot with exact Q' on GpSimd -> s12 (true scale, exact to ~2^-22).
  F. top-8 of s12 + softmax + indirect-DMA gather of V rows (4KB) + fused
     weighted accumulate -> out.
"""
import numpy as np

import concourse.bacc as bacc
import concourse.bass as bass
import concourse.mybir as mybir
from concourse.tile import TileContext
from concourse.bass_utils import run_bass_kernel_spmd

P = 128
D_MODEL = 1024
RANK = 128
N_COMPRESS = 16
N_KNOWLEDGE = 32768
K_TOP = 8
B, S = 2, 2048
N_CORES = 8
TOK_PER_CORE = (B * S) // N_CORES      # 512
N_TILES = TOK_PER_CORE // P            # 4
N_DC = D_MODEL // P                    # 8
N_G = 4                                # neuron groups of 4
CH = 1024                              # PSUM score chunk (2 banks)
SUP = 2048                             # max8 super-chunk
N_SUP = N_KNOWLEDGE // SUP             # 16
NCAND = N_SUP * 8                      # 128 packed candidates per tile
MARGIN = 12
SCALE = 1.0 / np.sqrt(np.float32(RANK))

# host prescales for the exact fp16-split matmuls (router + Q projection)
XS = 4.0
RWS = 32.0
WS = 32.0
QT_ACT_SCALE = float(SCALE / (XS * WS))      # QT = SCALE*Q from Q' = 128*Q
RT_EXP_SCALE = float(1.0 / (XS * RWS))       # router scores' = 128*rs
RESC_SCALE = float(SCALE / (XS * WS))        # s = RESC_SCALE * sum(K * Q')

BIG = 768.0                                  # rounds |s|<0.25 to 2^-14 grid
DELTA = float(2.0 ** -26)                    # index step in packed mantissa
IDELTA = float(2.0 ** 26)

f32 = mybir.dt.float32
f16 = mybir.dt.float16
u32 = mybir.dt.uint32


def _build():
    nc = bacc.Bacc("TRN2", target_bir_lowering=False, debug=False, num_devices=N_CORES)

    xh = nc.declare_dram_parameter("xh", [P, N_DC * TOK_PER_CORE], f16, isOutput=False)
    xl = nc.declare_dram_parameter("xl", [P, N_DC * TOK_PER_CORE], f16, isOutput=False)
    rwh = nc.declare_dram_parameter("rwh", [P, N_DC * N_COMPRESS], f16, isOutput=False)
    rwl = nc.declare_dram_parameter("rwl", [P, N_DC * N_COMPRESS], f16, isOutput=False)
    Wgh = nc.declare_dram_parameter("Wgh", [N_TILES * N_G * N_DC * P, 512], f16, isOutput=False)
    Wgl = nc.declare_dram_parameter("Wgl", [N_TILES * N_G * N_DC * P, 512], f16, isOutput=False)
    Kh = nc.declare_dram_parameter("Kh", [P, N_KNOWLEDGE], f16, isOutput=False)
    Krows = nc.declare_dram_parameter("Krows", [N_KNOWLEDGE, RANK], f32, isOutput=False)
    V = nc.declare_dram_parameter("V", [N_KNOWLEDGE, D_MODEL], f32, isOutput=False)
    iotaS = nc.declare_dram_parameter("iotaS", [P, SUP], f32, isOutput=False)
    baseS = nc.declare_dram_parameter("baseS", [P, NCAND], f32, isOutput=False)
    ident = nc.declare_dram_parameter("ident", [P, P], f32, isOutput=False)
    out = nc.declare_dram_parameter("out", [TOK_PER_CORE, D_MODEL], f32, isOutput=True)

    # Wg is replicated once per tile so B can stream tile-major:
    Wgh_v = Wgh.rearrange("(t g dc p) n -> t g dc p n", t=N_TILES, g=N_G, dc=N_DC)
    Wgl_v = Wgl.rearrange("(t g dc p) n -> t g dc p n", t=N_TILES, g=N_G, dc=N_DC)

    with TileContext(nc) as tc:
        with (
            tc.tile_pool(name="const", bufs=1) as cpool,
            tc.tile_pool(name="wld", bufs=3) as wpool,
            tc.tile_pool(name="s768", bufs=2) as spool768,
            tc.tile_pool(name="pack", bufs=2) as ppool,
            tc.tile_pool(name="cand", bufs=2) as candpool,
            tc.tile_pool(name="kg", bufs=2) as kgpool,
            tc.tile_pool(name="gat", bufs=1) as gpool,
            tc.tile_pool(name="acc", bufs=2) as apool,
            tc.tile_pool(name="small", bufs=6) as spool,
            tc.tile_pool(name="ps_sc", bufs=2, space="PSUM") as ps_sc,
            tc.tile_pool(name="ps_y", bufs=2, space="PSUM") as ps_y,
            tc.tile_pool(name="ps_sm", bufs=1, space="PSUM") as ps_sm,
        ):
            # ---- persistent loads ----
            xh_sb = cpool.tile([P, N_DC * TOK_PER_CORE], f16)
            xl_sb = cpool.tile([P, N_DC * TOK_PER_CORE], f16)
            rwh_sb = cpool.tile([P, N_DC * N_COMPRESS], f16)
            rwl_sb = cpool.tile([P, N_DC * N_COMPRESS], f16)
            id_sb = cpool.tile([P, P], f32)
            kh_sb = cpool.tile([P, N_KNOWLEDGE], f16)       # 32KB/part resident
            iota_sb = cpool.tile([P, SUP], f32)
            base_sb = cpool.tile([P, NCAND], f32)
            nc.sync.dma_start(out=xh_sb[:], in_=xh[:])
            nc.sync.dma_start(out=xl_sb[:], in_=xl[:])
            nc.sync.dma_start(out=rwh_sb[:], in_=rwh[:])
            nc.sync.dma_start(out=rwl_sb[:], in_=rwl[:])
            nc.sync.dma_start(out=id_sb[:], in_=ident[:])
            nc.sync.dma_start(out=kh_sb[:], in_=Kh[:])
            nc.sync.dma_start(out=iota_sb[:], in_=iotaS[:])
            nc.sync.dma_start(out=base_sb[:], in_=baseS[:])

            wts_sb = cpool.tile([P, N_TILES * N_COMPRESS], f32)
            Q_sb = cpool.tile([P, N_TILES * RANK], f32)      # Q' = 128*Q, exact
            QT_sb = cpool.tile([P, N_TILES * P], f32)        # SCALE*Q
            QTh_sb = cpool.tile([P, N_TILES * P], f16)       # screen lhsT

            def tok(t):
                return slice(t * P, (t + 1) * P)

            cand_t = {}
            gixf_t = {}
            gix12_t = {}
            kg_t = {}
            w8_t = {}
            gidx8_t = {}
            gat_t = {}

            def stage_abc(t):
                # ---- A: router softmax (fp16 2-term, exact) ----
                rps = ps_sm.tile([P, N_COMPRESS], f32, space="PSUM", tag="rps")
                n_mm = N_DC * 3
                i_mm = 0
                for dc in range(N_DC):
                    xsl = slice(dc * TOK_PER_CORE + t * P, dc * TOK_PER_CORE + (t + 1) * P)
                    rsl = slice(dc * N_COMPRESS, (dc + 1) * N_COMPRESS)
                    for lhs, rhs in ((xh_sb, rwh_sb), (xh_sb, rwl_sb), (xl_sb, rwh_sb)):
                        nc.tensor.matmul(out=rps[:], lhsT=lhs[:, xsl], rhs=rhs[:, rsl],
                                         start=(i_mm == 0), stop=(i_mm == n_mm - 1))
                        i_mm += 1
                w = wts_sb[:, t * N_COMPRESS:(t + 1) * N_COMPRESS]
                mx = spool.tile([P, 1], f32, tag="mx")
                sm = spool.tile([P, 1], f32, tag="sm")
                ex = spool.tile([P, N_COMPRESS], f32, tag="ex")
                nc.vector.tensor_reduce(out=mx[:], in_=rps[:], op=mybir.AluOpType.max, axis=mybir.AxisListType.X)
                nc.vector.tensor_scalar(out=ex[:], in0=rps[:], scalar1=mx[:, :1], scalar2=None, op0=mybir.AluOpType.subtract)
                nc.scalar.activation(out=ex[:], in_=ex[:], func=mybir.ActivationFunctionType.Exp,
                                     scale=RT_EXP_SCALE, accum_out=sm[:, :1])
                rcp = spool.tile([P, 1], f32, tag="rcp")
                nc.vector.reciprocal(out=rcp[:], in_=sm[:, :1])
                nc.vector.tensor_scalar(out=w, in0=ex[:], scalar1=rcp[:, :1], scalar2=None, op0=mybir.AluOpType.mult)

                # ---- B: exact Q' (fp16 3-term), Wg streamed tile-major ----
                q = Q_sb[:, t * RANK:(t + 1) * RANK]
                for g in range(N_G):
                    yps = ps_y.tile([P, 512], f32, space="PSUM", tag="yps")
                    for dc in range(N_DC):
                        wh = wpool.tile([P, 512], f16, tag="wldh")
                        wl = wpool.tile([P, 512], f16, tag="wldl")
                        nc.sync.dma_start(out=wh[:], in_=Wgh_v[t, g, dc])
                        nc.gpsimd.dma_start(out=wl[:], in_=Wgl_v[t, g, dc])
                        xsl = slice(dc * TOK_PER_CORE + t * P, dc * TOK_PER_CORE + (t + 1) * P)
                        for j, (lhs, rhs) in enumerate(((xh_sb, wh), (xh_sb, wl), (xl_sb, wh))):
                            nc.tensor.matmul(out=yps[:], lhsT=lhs[:, xsl], rhs=rhs[:],
                                             start=(dc == 0 and j == 0),
                                             stop=(dc == N_DC - 1 and j == 2))
                    for n in range(4):
                        ncomp = g * 4 + n
                        wcol = wts_sb[:, t * N_COMPRESS + ncomp:t * N_COMPRESS + ncomp + 1]
                        ypart = yps[:, n * RANK:(n + 1) * RANK]
                        if g == 0 and n == 0:
                            nc.vector.tensor_scalar(out=q, in0=ypart, scalar1=wcol, scalar2=None,
                                                    op0=mybir.AluOpType.mult)
                        else:
                            nc.vector.scalar_tensor_tensor(out=q, in0=ypart, scalar=wcol, in1=q,
                                                           op0=mybir.AluOpType.mult,
                                                           op1=mybir.AluOpType.add)

                # ---- T: transpose -> QT (true scale), screen cast ----
                tps = ps_sm.tile([P, P], f32, space="PSUM", tag="tps")
                nc.tensor.transpose(out=tps[:], in_=Q_sb[:, t * RANK:(t + 1) * RANK], identity=id_sb[:])
                nc.scalar.activation(out=QT_sb[:, tok(t)], in_=tps[:],
                                     func=mybir.ActivationFunctionType.Copy, scale=QT_ACT_SCALE)
                nc.vector.tensor_copy(out=QTh_sb[:, tok(t)], in_=QT_sb[:, tok(t)])

                # ---- C: screen scores, pack, per-super-chunk max8 ----
                cand = candpool.tile([P, NCAND], f32, tag="cand")
                cand_t[t] = cand
                for sup in range(N_SUP):
                    s768 = spool768.tile([P, SUP], f32, tag="s768")
                    pk = ppool.tile([P, SUP], f32, tag="pk")
                    for h in range(2):
                        sps = ps_sc.tile([P, CH], f32, space="PSUM", tag="sc")
                        for hh in range(2):
                            ksl = slice(sup * SUP + h * CH + hh * 512,
                                        sup * SUP + h * CH + (hh + 1) * 512)
                            nc.tensor.matmul(out=sps[:, hh * 512:(hh + 1) * 512],
                                             lhsT=QTh_sb[:, tok(t)], rhs=kh_sb[:, ksl],
                                             start=True, stop=True)
                        nc.scalar.activation(out=s768[:, h * CH:(h + 1) * CH], in_=sps[:],
                                             func=mybir.ActivationFunctionType.Copy, bias=BIG)
                        nc.vector.scalar_tensor_tensor(
                            out=pk[:, h * CH:(h + 1) * CH], in0=s768[:, h * CH:(h + 1) * CH],
                            scalar=-BIG, in1=iota_sb[:, h * CH:(h + 1) * CH],
                            op0=mybir.AluOpType.add, op1=mybir.AluOpType.add)
                    nc.vector.max(out=cand[:, sup * 8:(sup + 1) * 8], in_=pk[:])

            def stage_da(t):
                # ---- Da: merge to top-12 packed, unpack, resolve, K-gather ----
                cand = cand_t[t]
                top8 = spool.tile([P, 8], f32, tag="top8")
                nxt8 = spool.tile([P, 8], f32, tag="nxt8")
                cmr = candpool.tile([P, NCAND], f32, tag="cmr")
                nc.vector.max(out=top8[:], in_=cand[:])
                nc.vector.match_replace(out=cmr[:], in_to_replace=top8[:], in_values=cand[:],
                                        imm_value=-1e30)
                nc.vector.max(out=nxt8[:], in_=cmr[:])
                pk12 = spool.tile([P, MARGIN], f32, tag="pk12")
                nc.vector.tensor_copy(out=pk12[:, 0:8], in_=top8[:])
                nc.vector.tensor_copy(out=pk12[:, 8:MARGIN], in_=nxt8[:, 0:MARGIN - 8])
                q12 = spool.tile([P, MARGIN], f32, tag="q12")
                nc.vector.tensor_scalar(out=q12[:], in0=pk12[:], scalar1=BIG, scalar2=None,
                                        op0=mybir.AluOpType.add)
                nc.vector.tensor_scalar(out=q12[:], in0=q12[:], scalar1=BIG, scalar2=None,
                                        op0=mybir.AluOpType.subtract)
                lidx = spool.tile([P, MARGIN], f32, tag="lidx")
                nc.vector.tensor_tensor(out=lidx[:], in0=pk12[:], in1=q12[:], op=mybir.AluOpType.subtract)
                nc.vector.tensor_scalar(out=lidx[:], in0=lidx[:], scalar1=IDELTA, scalar2=None,
                                        op0=mybir.AluOpType.mult)
                gixf = spool.tile([P, MARGIN], f32, tag="gixf")
                gixf_t[t] = gixf
                junk = spool.tile([P, NCAND], f32, tag="junk")
                for j in range(MARGIN):
                    nc.vector.scalar_tensor_tensor(
                        out=junk[:], in0=cand[:], scalar=pk12[:, j:j + 1], in1=base_sb[:],
                        op0=mybir.AluOpType.is_equal, op1=mybir.AluOpType.mult,
                        accum_out=gixf[:, j:j + 1])
                nc.vector.tensor_tensor(out=gixf[:], in0=gixf[:], in1=lidx[:], op=mybir.AluOpType.add)
                gix12 = spool.tile([P, MARGIN], u32, tag="gix12")
                gix12_t[t] = gix12
                nc.vector.tensor_copy(out=gix12[:], in_=gixf[:])
                kg = kgpool.tile([P, MARGIN * RANK], f32, tag="kg")
                kg_t[t] = kg
                for j in range(MARGIN):
                    nc.gpsimd.indirect_dma_start(
                        out=kg[:, j * RANK:(j + 1) * RANK], out_offset=None, in_=Krows[:],
                        in_offset=bass.IndirectOffsetOnAxis(ap=gix12[:, j:j + 1], axis=0))

            def stage_db(t):
                # ---- Db: exact rescore, top-8, softmax, V-gather ----
                kg = kg_t[t]
                gixf = gixf_t[t]
                s12 = spool.tile([P, MARGIN], f32, tag="s12")
                junk2 = spool.tile([P, RANK], f32, tag="junk2")
                for j in range(MARGIN):
                    nc.vector.scalar_tensor_tensor(
                        out=junk2[:], in0=kg[:, j * RANK:(j + 1) * RANK], scalar=RESC_SCALE,
                        in1=Q_sb[:, t * RANK:(t + 1) * RANK],
                        op0=mybir.AluOpType.mult, op1=mybir.AluOpType.mult,
                        accum_out=s12[:, j:j + 1])
                v8 = spool.tile([P, 8], f32, tag="v8")
                nc.vector.max(out=v8[:], in_=s12[:])
                gf8 = spool.tile([P, 8], f32, tag="gf8")
                junk3 = spool.tile([P, MARGIN], f32, tag="junk3")
                for j in range(K_TOP):
                    nc.vector.scalar_tensor_tensor(
                        out=junk3[:], in0=s12[:], scalar=v8[:, j:j + 1], in1=gixf[:],
                        op0=mybir.AluOpType.is_equal, op1=mybir.AluOpType.mult,
                        accum_out=gf8[:, j:j + 1])
                gidx8 = spool.tile([P, 8], u32, tag="gidx8")
                gidx8_t[t] = gidx8
                nc.vector.tensor_copy(out=gidx8[:], in_=gf8[:])

                w8 = spool.tile([P, 8], f32, tag="w8")
                w8_t[t] = w8
                sm8 = spool.tile([P, 1], f32, tag="sm8")
                nc.vector.tensor_scalar(out=w8[:], in0=v8[:], scalar1=v8[:, :1], scalar2=None,
                                        op0=mybir.AluOpType.subtract)
                nc.scalar.activation(out=w8[:], in_=w8[:], func=mybir.ActivationFunctionType.Exp,
                                     accum_out=sm8[:, :1])
                rcp8 = spool.tile([P, 1], f32, tag="rcp8")
                nc.vector.reciprocal(out=rcp8[:], in_=sm8[:, :1])
                nc.vector.tensor_scalar(out=w8[:], in0=w8[:], scalar1=rcp8[:, :1], scalar2=None,
                                        op0=mybir.AluOpType.mult)

                gat = gpool.tile([P, K_TOP * D_MODEL], f32, tag="gat")
                gat_t[t] = gat
                for j in range(K_TOP):
                    nc.gpsimd.indirect_dma_start(
                        out=gat[:, j * D_MODEL:(j + 1) * D_MODEL], out_offset=None, in_=V[:],
                        in_offset=bass.IndirectOffsetOnAxis(ap=gidx8[:, j:j + 1], axis=0))

            def stage_dc(t):
                # ---- Dc: weighted accumulate + store ----
                gat = gat_t[t]
                w8 = w8_t[t]
                acc = apool.tile([P, D_MODEL], f32, tag="acc")
                nc.vector.tensor_scalar(out=acc[:], in0=gat[:, 0:D_MODEL], scalar1=w8[:, 0:1],
                                        scalar2=None, op0=mybir.AluOpType.mult)
                for j in range(1, K_TOP):
                    nc.vector.scalar_tensor_tensor(
                        out=acc[:], in0=gat[:, j * D_MODEL:(j + 1) * D_MODEL], scalar=w8[:, j:j + 1],
                        in1=acc[:], op0=mybir.AluOpType.mult, op1=mybir.AluOpType.add)
                nc.sync.dma_start(out=out[t * P:(t + 1) * P, :], in_=acc[:])

            # software pipeline: gathers of tile t run during tile t+1's scoring
            stage_abc(0); stage_da(0)
            stage_abc(1); stage_da(1); stage_db(0)
            stage_abc(2); stage_da(2); stage_dc(0); stage_db(1)
            stage_abc(3); stage_da(3); stage_dc(1); stage_db(2)
            stage_dc(2); stage_db(3); stage_dc(3)

    nc.compile()
    return nc


_NC_CACHE = {}


def _get_nc():
    if "v4" not in _NC_CACHE:
        _NC_CACHE["v4"] = _build()
    return _NC_CACHE["v4"]


def _split16(a):
    hi = a.astype(np.float16)
    lo = (a - hi.astype(np.float32)).astype(np.float16)
    return hi, lo


def _prep_in_maps(x, router_w, compress_neurons, knowledge_K, knowledge_V):
    x = np.asarray(x, dtype=np.float32).reshape(B * S, D_MODEL) * XS
    rwT = np.ascontiguousarray(np.asarray(router_w, dtype=np.float32).T) * RWS
    rw_r = np.ascontiguousarray(
        rwT.reshape(N_DC, P, N_COMPRESS).transpose(1, 0, 2).reshape(P, N_DC * N_COMPRESS))
    rwh, rwl = _split16(rw_r)
    cn = np.asarray(compress_neurons, dtype=np.float32) * WS
    Wg1 = np.ascontiguousarray(
        cn.reshape(N_G, 4, N_DC, P, RANK).transpose(0, 2, 3, 1, 4).reshape(N_G * N_DC * P, 4 * RANK))
    Wg = np.ascontiguousarray(np.tile(Wg1, (N_TILES, 1)))   # tile-major replication
    Wgh, Wgl = _split16(Wg)
    K = np.asarray(knowledge_K, dtype=np.float32)
    KT = np.ascontiguousarray(K.T)                           # [128, 32768]
    Kh = KT.astype(np.float16)
    Vf = np.ascontiguousarray(np.asarray(knowledge_V, dtype=np.float32))
    ident = np.eye(P, dtype=np.float32)
    iota = np.tile((np.arange(SUP, dtype=np.float64) * (2.0 ** -26)).astype(np.float32), (P, 1))
    base = np.tile(((np.arange(NCAND) // 8) * SUP).astype(np.float32), (P, 1))
    Krows_f = np.ascontiguousarray(K)

    in_maps = []
    for c in range(N_CORES):
        xs = x[c * TOK_PER_CORE:(c + 1) * TOK_PER_CORE]
        xT = np.ascontiguousarray(
            xs.T.reshape(N_DC, P, TOK_PER_CORE).transpose(1, 0, 2).reshape(P, N_DC * TOK_PER_CORE))
        xhc, xlc = _split16(xT)
        in_maps.append(dict(xh=xhc, xl=xlc, rwh=rwh, rwl=rwl, Wgh=Wgh, Wgl=Wgl,
                            Kh=Kh, Krows=Krows_f, V=Vf, iotaS=iota, baseS=base,
                            ident=ident))
    return in_maps


def _ensure_ntff_hook():
    import sys as _sys
    import types as _types
    if "antenv.axon_hooks" in _sys.modules:
        return
    try:
        import antenv.axon_hooks  # noqa: F401
        return
    except ImportError:
        pass
    mod = _types.ModuleType("antenv.axon_hooks")
    _state = {"hook": None}
    mod.set_axon_ntff_profile_hook = lambda h: _state.__setitem__("hook", h)
    mod.get_axon_ntff_profile_hook = lambda: _state["hook"]
    _sys.modules["antenv.axon_hooks"] = mod
    try:
        from trn_agent_boot.trn_boot import _ntff_profile_via_ctypes
        mod.set_axon_ntff_profile_hook(_ntff_profile_via_ctypes("/opt/axon/libaxon_pjrt.so"))
    except Exception:
        pass


def _run(inputs, trace=False, dbg=False):
    if trace:
        _ensure_ntff_hook()
    nc = _get_nc()
    in_maps = _prep_in_maps(**inputs)
    res = run_bass_kernel_spmd(nc, in_maps, core_ids=list(range(N_CORES)), trace=trace)
    out = np.concatenate([res.results[c]["out"] for c in range(N_CORES)], axis=0)
    return out.reshape(B, S, D_MODEL), res


def kernel(x, router_w, compress_neurons, knowledge_K, knowledge_V):
    out, _ = _run(dict(x=x, router_w=router_w, compress_neurons=compress_neurons,
                       knowledge_K=knowledge_K, knowledge_V=knowledge_V))
    return out


# revision 20
# speedup vs baseline: 1.0491x; 1.0491x over previous
"""NeuronMemory retrieval kernel for 8 TRN2 NeuronCores — v4.

Per token: softmax-routed low-rank projection Q (rank 128), dense scores
against 32768 knowledge keys, top-8, softmax, weighted gather of V rows.
Sharding: data-parallel over the 4096 tokens (512/core); tables replicated.

Architecture (per core, 4 token tiles of 128):
  A. router scores (fp16 2-term split, prescaled) + softmax -> wts
  B. Q' = 128*Q via fp16 3-term-split matmuls + fused weighting (exact to
     ~2^-22); Wg streamed per tile so tile t+1's B overlaps tile t's C.
  T. PE-transpose -> QT (true scale SCALE*Q) -> QTh = fp16(QT) for screening
  C. screening scores s ~= QTh.T @ Kh (fp16 1-term, resident Kh) -> PSUM
     per 1024-chunk; Scalar copies +768 (rounds mantissa to 2^-14 grid);
     Vector packs p = (s768-768) + idx*2^-26 (chunk-local index embedded in
     low mantissa bits); Vector max8 per 2048 super-chunk -> 8 packed
     candidates each (16 super-chunks -> 128 candidates, provably contains
     the true top-8 up to fp16 screen noise ~3e-4 rel, covered by margin).
  D. merge: max8 + match_replace + max8 -> top-12 packed finalists; unpack
     value+index; resolve super-chunk base via is_equal one-hot dot with a
     per-slot base table.
  E. exact rescore: indirect-DMA gather the 12 candidate K rows (fp32) and
     dot with exact Q' on GpSimd -> s12 (true scale, exact to ~2^-22).
  F. top-8 of s12 + softmax + indirect-DMA gather of V rows (4KB) + fused
     weighted accumulate -> out.
"""
import numpy as np

import concourse.bacc as bacc
import concourse.bass as bass
import concourse.mybir as mybir
from concourse.tile import TileContext
from concourse.bass_utils import run_bass_kernel_spmd

P = 128
D_MODEL = 1024
RANK = 128
N_COMPRESS = 16
N_KNOWLEDGE = 32768
K_TOP = 8
B, S = 2, 2048
N_CORES = 8
TOK_PER_CORE = (B * S) // N_CORES      # 512
N_TILES = TOK_PER_CORE // P            # 4
N_DC = D_MODEL // P                    # 8
N_G = 4                                # neuron groups of 4
CH = 1024                              # PSUM score chunk (2 banks)
SUP = 2048                             # max8 super-chunk
N_SUP = N_KNOWLEDGE // SUP             # 16
NCAND = N_SUP * 8                      # 128 packed candidates per tile
MARGIN = 12
SCALE = 1.0 / np.sqrt(np.float32(RANK))

# host prescales for the exact fp16-split matmuls (router + Q projection)
XS = 4.0
RWS = 32.0
WS = 32.0
QT_ACT_SCALE = float(SCALE / (XS * WS))      # QT = SCALE*Q from Q' = 128*Q
RT_EXP_SCALE = float(1.0 / (XS * RWS))       # router scores' = 128*rs
RESC_SCALE = float(SCALE / (XS * WS))        # s = RESC_SCALE * sum(K * Q')

BIG = 768.0                                  # rounds |s|<0.25 to 2^-14 grid
DELTA = float(2.0 ** -26)                    # index step in packed mantissa
IDELTA = float(2.0 ** 26)

f32 = mybir.dt.float32
f16 = mybir.dt.float16
u32 = mybir.dt.uint32


def _build():
    nc = bacc.Bacc("TRN2", target_bir_lowering=False, debug=False, num_devices=N_CORES)

    xh = nc.declare_dram_parameter("xh", [P, N_DC * TOK_PER_CORE], f16, isOutput=False)
    xl = nc.declare_dram_parameter("xl", [P, N_DC * TOK_PER_CORE], f16, isOutput=False)
    rwh = nc.declare_dram_parameter("rwh", [P, N_DC * N_COMPRESS], f16, isOutput=False)
    rwl = nc.declare_dram_parameter("rwl", [P, N_DC * N_COMPRESS], f16, isOutput=False)
    Wgh = nc.declare_dram_parameter("Wgh", [N_TILES * N_G * N_DC * P, 512], f16, isOutput=False)
    Wgl = nc.declare_dram_parameter("Wgl", [N_TILES * N_G * N_DC * P, 512], f16, isOutput=False)
    Kh = nc.declare_dram_parameter("Kh", [P, N_KNOWLEDGE], f16, isOutput=False)
    Krows = nc.declare_dram_parameter("Krows", [N_KNOWLEDGE, RANK], f32, isOutput=False)
    V = nc.declare_dram_parameter("V", [N_KNOWLEDGE, D_MODEL], f32, isOutput=False)
    iotaS = nc.declare_dram_parameter("iotaS", [P, SUP], f32, isOutput=False)
    baseS = nc.declare_dram_parameter("baseS", [P, NCAND], f32, isOutput=False)
    ident = nc.declare_dram_parameter("ident", [P, P], f32, isOutput=False)
    out = nc.declare_dram_parameter("out", [TOK_PER_CORE, D_MODEL], f32, isOutput=True)

    # Wg is replicated once per tile so B can stream tile-major:
    Wgh_v = Wgh.rearrange("(t g dc p) n -> t g dc p n", t=N_TILES, g=N_G, dc=N_DC)
    Wgl_v = Wgl.rearrange("(t g dc p) n -> t g dc p n", t=N_TILES, g=N_G, dc=N_DC)

    with TileContext(nc) as tc:
        with (
            tc.tile_pool(name="const", bufs=1) as cpool,
            tc.tile_pool(name="wld", bufs=6) as wpool,
            tc.tile_pool(name="s768", bufs=2) as spool768,
            tc.tile_pool(name="pack", bufs=2) as ppool,
            tc.tile_pool(name="cand", bufs=2) as candpool,
            tc.tile_pool(name="kg", bufs=2) as kgpool,
            tc.tile_pool(name="gat", bufs=1) as gpool,
            tc.tile_pool(name="acc", bufs=2) as apool,
            tc.tile_pool(name="small", bufs=6) as spool,
            tc.tile_pool(name="ps_sc", bufs=2, space="PSUM") as ps_sc,
            tc.tile_pool(name="ps_y", bufs=2, space="PSUM") as ps_y,
            tc.tile_pool(name="ps_sm", bufs=1, space="PSUM") as ps_sm,
        ):
            # ---- persistent loads ----
            xh_sb = cpool.tile([P, N_DC * TOK_PER_CORE], f16)
            xl_sb = cpool.tile([P, N_DC * TOK_PER_CORE], f16)
            rwh_sb = cpool.tile([P, N_DC * N_COMPRESS], f16)
            rwl_sb = cpool.tile([P, N_DC * N_COMPRESS], f16)
            id_sb = cpool.tile([P, P], f32)
            kh_sb = cpool.tile([P, N_KNOWLEDGE], f16)       # 32KB/part resident
            iota_sb = cpool.tile([P, SUP], f32)
            base_sb = cpool.tile([P, NCAND], f32)
            nc.sync.dma_start(out=xh_sb[:], in_=xh[:])
            nc.sync.dma_start(out=xl_sb[:], in_=xl[:])
            nc.sync.dma_start(out=rwh_sb[:], in_=rwh[:])
            nc.sync.dma_start(out=rwl_sb[:], in_=rwl[:])
            nc.sync.dma_start(out=id_sb[:], in_=ident[:])
            nc.sync.dma_start(out=kh_sb[:], in_=Kh[:])
            nc.sync.dma_start(out=iota_sb[:], in_=iotaS[:])
            nc.sync.dma_start(out=base_sb[:], in_=baseS[:])

            wts_sb = cpool.tile([P, N_TILES * N_COMPRESS], f32)
            Q_sb = cpool.tile([P, N_TILES * RANK], f32)      # Q' = 128*Q, exact
            QT_sb = cpool.tile([P, N_TILES * P], f32)        # SCALE*Q
            QTh_sb = cpool.tile([P, N_TILES * P], f16)       # screen lhsT

            def tok(t):
                return slice(t * P, (t + 1) * P)

            cand_t = {}
            gixf_t = {}
            gix12_t = {}
            kg_t = {}
            w8_t = {}
            gidx8_t = {}
            gat_t = {}

            def stage_ab(t):
                # ---- A: router softmax (fp16 2-term, exact) ----
                rps = ps_sm.tile([P, N_COMPRESS], f32, space="PSUM", tag="rps")
                n_mm = N_DC * 3
                i_mm = 0
                for dc in range(N_DC):
                    xsl = slice(dc * TOK_PER_CORE + t * P, dc * TOK_PER_CORE + (t + 1) * P)
                    rsl = slice(dc * N_COMPRESS, (dc + 1) * N_COMPRESS)
                    for lhs, rhs in ((xh_sb, rwh_sb), (xh_sb, rwl_sb), (xl_sb, rwh_sb)):
                        nc.tensor.matmul(out=rps[:], lhsT=lhs[:, xsl], rhs=rhs[:, rsl],
                                         start=(i_mm == 0), stop=(i_mm == n_mm - 1))
                        i_mm += 1
                w = wts_sb[:, t * N_COMPRESS:(t + 1) * N_COMPRESS]
                mx = spool.tile([P, 1], f32, tag="mx")
                sm = spool.tile([P, 1], f32, tag="sm")
                ex = spool.tile([P, N_COMPRESS], f32, tag="ex")
                nc.vector.tensor_reduce(out=mx[:], in_=rps[:], op=mybir.AluOpType.max, axis=mybir.AxisListType.X)
                nc.vector.tensor_scalar(out=ex[:], in0=rps[:], scalar1=mx[:, :1], scalar2=None, op0=mybir.AluOpType.subtract)
                nc.scalar.activation(out=ex[:], in_=ex[:], func=mybir.ActivationFunctionType.Exp,
                                     scale=RT_EXP_SCALE, accum_out=sm[:, :1])
                rcp = spool.tile([P, 1], f32, tag="rcp")
                nc.vector.reciprocal(out=rcp[:], in_=sm[:, :1])
                nc.vector.tensor_scalar(out=w, in0=ex[:], scalar1=rcp[:, :1], scalar2=None, op0=mybir.AluOpType.mult)

                # ---- B: exact Q' (fp16 3-term), Wg streamed tile-major ----
                q = Q_sb[:, t * RANK:(t + 1) * RANK]
                for g in range(N_G):
                    yps = ps_y.tile([P, 512], f32, space="PSUM", tag="yps")
                    for dc in range(N_DC):
                        wh = wpool.tile([P, 512], f16, tag="wldh")
                        wl = wpool.tile([P, 512], f16, tag="wldl")
                        nc.sync.dma_start(out=wh[:], in_=Wgh_v[t, g, dc])
                        nc.sync.dma_start(out=wl[:], in_=Wgl_v[t, g, dc])
                        xsl = slice(dc * TOK_PER_CORE + t * P, dc * TOK_PER_CORE + (t + 1) * P)
                        for j, (lhs, rhs) in enumerate(((xh_sb, wh), (xh_sb, wl), (xl_sb, wh))):
                            nc.tensor.matmul(out=yps[:], lhsT=lhs[:, xsl], rhs=rhs[:],
                                             start=(dc == 0 and j == 0),
                                             stop=(dc == N_DC - 1 and j == 2))
                    for n in range(4):
                        ncomp = g * 4 + n
                        wcol = wts_sb[:, t * N_COMPRESS + ncomp:t * N_COMPRESS + ncomp + 1]
                        ypart = yps[:, n * RANK:(n + 1) * RANK]
                        if g == 0 and n == 0:
                            nc.vector.tensor_scalar(out=q, in0=ypart, scalar1=wcol, scalar2=None,
                                                    op0=mybir.AluOpType.mult)
                        else:
                            nc.vector.scalar_tensor_tensor(out=q, in0=ypart, scalar=wcol, in1=q,
                                                           op0=mybir.AluOpType.mult,
                                                           op1=mybir.AluOpType.add)

                # ---- T: transpose -> QT (true scale), screen cast ----
                tps = ps_sm.tile([P, P], f32, space="PSUM", tag="tps")
                nc.tensor.transpose(out=tps[:], in_=Q_sb[:, t * RANK:(t + 1) * RANK], identity=id_sb[:])
                nc.scalar.activation(out=QT_sb[:, tok(t)], in_=tps[:],
                                     func=mybir.ActivationFunctionType.Copy, scale=QT_ACT_SCALE)
                nc.vector.tensor_copy(out=QTh_sb[:, tok(t)], in_=QT_sb[:, tok(t)])

            def stage_c(t):
                # ---- C: screen scores, pack, per-super-chunk max8 ----
                cand = candpool.tile([P, NCAND], f32, tag="cand")
                cand_t[t] = cand
                for sup in range(N_SUP):
                    s768 = spool768.tile([P, SUP], f32, tag="s768")
                    pk = ppool.tile([P, SUP], f32, tag="pk")
                    for h in range(2):
                        sps = ps_sc.tile([P, CH], f32, space="PSUM", tag="sc")
                        for hh in range(2):
                            ksl = slice(sup * SUP + h * CH + hh * 512,
                                        sup * SUP + h * CH + (hh + 1) * 512)
                            nc.tensor.matmul(out=sps[:, hh * 512:(hh + 1) * 512],
                                             lhsT=QTh_sb[:, tok(t)], rhs=kh_sb[:, ksl],
                                             start=True, stop=True)
                        nc.scalar.activation(out=s768[:, h * CH:(h + 1) * CH], in_=sps[:],
                                             func=mybir.ActivationFunctionType.Copy, bias=BIG)
                        nc.vector.scalar_tensor_tensor(
                            out=pk[:, h * CH:(h + 1) * CH], in0=s768[:, h * CH:(h + 1) * CH],
                            scalar=-BIG, in1=iota_sb[:, h * CH:(h + 1) * CH],
                            op0=mybir.AluOpType.add, op1=mybir.AluOpType.add)
                    nc.vector.max(out=cand[:, sup * 8:(sup + 1) * 8], in_=pk[:])

            def stage_da(t):
                # ---- Da: merge to top-12 packed, unpack, resolve, K-gather ----
                cand = cand_t[t]
                top8 = spool.tile([P, 8], f32, tag="top8")
                nxt8 = spool.tile([P, 8], f32, tag="nxt8")
                cmr = candpool.tile([P, NCAND], f32, tag="cmr")
                nc.vector.max(out=top8[:], in_=cand[:])
                nc.vector.match_replace(out=cmr[:], in_to_replace=top8[:], in_values=cand[:],
                                        imm_value=-1e30)
                nc.vector.max(out=nxt8[:], in_=cmr[:])
                pk12 = spool.tile([P, MARGIN], f32, tag="pk12")
                nc.vector.tensor_copy(out=pk12[:, 0:8], in_=top8[:])
                nc.vector.tensor_copy(out=pk12[:, 8:MARGIN], in_=nxt8[:, 0:MARGIN - 8])
                q12 = spool.tile([P, MARGIN], f32, tag="q12")
                nc.vector.tensor_scalar(out=q12[:], in0=pk12[:], scalar1=BIG, scalar2=None,
                                        op0=mybir.AluOpType.add)
                nc.vector.tensor_scalar(out=q12[:], in0=q12[:], scalar1=BIG, scalar2=None,
                                        op0=mybir.AluOpType.subtract)
                lidx = spool.tile([P, MARGIN], f32, tag="lidx")
                nc.vector.tensor_tensor(out=lidx[:], in0=pk12[:], in1=q12[:], op=mybir.AluOpType.subtract)
                nc.vector.tensor_scalar(out=lidx[:], in0=lidx[:], scalar1=IDELTA, scalar2=None,
                                        op0=mybir.AluOpType.mult)
                gixf = spool.tile([P, MARGIN], f32, tag="gixf")
                gixf_t[t] = gixf
                junk = spool.tile([P, NCAND], f32, tag="junk")
                for j in range(MARGIN):
                    nc.vector.scalar_tensor_tensor(
                        out=junk[:], in0=cand[:], scalar=pk12[:, j:j + 1], in1=base_sb[:],
                        op0=mybir.AluOpType.is_equal, op1=mybir.AluOpType.mult,
                        accum_out=gixf[:, j:j + 1])
                nc.vector.tensor_tensor(out=gixf[:], in0=gixf[:], in1=lidx[:], op=mybir.AluOpType.add)
                gix12 = spool.tile([P, MARGIN], u32, tag="gix12")
                gix12_t[t] = gix12
                nc.vector.tensor_copy(out=gix12[:], in_=gixf[:])
                kg = kgpool.tile([P, MARGIN * RANK], f32, tag="kg")
                kg_t[t] = kg
                for j in range(MARGIN):
                    nc.gpsimd.indirect_dma_start(
                        out=kg[:, j * RANK:(j + 1) * RANK], out_offset=None, in_=Krows[:],
                        in_offset=bass.IndirectOffsetOnAxis(ap=gix12[:, j:j + 1], axis=0))

            def stage_db(t):
                # ---- Db: exact rescore, top-8, softmax, V-gather ----
                kg = kg_t[t]
                gixf = gixf_t[t]
                s12 = spool.tile([P, MARGIN], f32, tag="s12")
                junk2 = spool.tile([P, RANK], f32, tag="junk2")
                for j in range(MARGIN):
                    nc.vector.scalar_tensor_tensor(
                        out=junk2[:], in0=kg[:, j * RANK:(j + 1) * RANK], scalar=RESC_SCALE,
                        in1=Q_sb[:, t * RANK:(t + 1) * RANK],
                        op0=mybir.AluOpType.mult, op1=mybir.AluOpType.mult,
                        accum_out=s12[:, j:j + 1])
                v8 = spool.tile([P, 8], f32, tag="v8")
                nc.vector.max(out=v8[:], in_=s12[:])
                gf8 = spool.tile([P, 8], f32, tag="gf8")
                junk3 = spool.tile([P, MARGIN], f32, tag="junk3")
                for j in range(K_TOP):
                    nc.vector.scalar_tensor_tensor(
                        out=junk3[:], in0=s12[:], scalar=v8[:, j:j + 1], in1=gixf[:],
                        op0=mybir.AluOpType.is_equal, op1=mybir.AluOpType.mult,
                        accum_out=gf8[:, j:j + 1])
                gidx8 = spool.tile([P, 8], u32, tag="gidx8")
                gidx8_t[t] = gidx8
                nc.vector.tensor_copy(out=gidx8[:], in_=gf8[:])

                w8 = spool.tile([P, 8], f32, tag="w8")
                w8_t[t] = w8
                sm8 = spool.tile([P, 1], f32, tag="sm8")
                nc.vector.tensor_scalar(out=w8[:], in0=v8[:], scalar1=v8[:, :1], scalar2=None,
                                        op0=mybir.AluOpType.subtract)
                nc.scalar.activation(out=w8[:], in_=w8[:], func=mybir.ActivationFunctionType.Exp,
                                     accum_out=sm8[:, :1])
                rcp8 = spool.tile([P, 1], f32, tag="rcp8")
                nc.vector.reciprocal(out=rcp8[:], in_=sm8[:, :1])
                nc.vector.tensor_scalar(out=w8[:], in0=w8[:], scalar1=rcp8[:, :1], scalar2=None,
                                        op0=mybir.AluOpType.mult)

                gat = gpool.tile([P, K_TOP * D_MODEL], f32, tag="gat")
                gat_t[t] = gat
                for j in range(K_TOP):
                    nc.gpsimd.indirect_dma_start(
                        out=gat[:, j * D_MODEL:(j + 1) * D_MODEL], out_offset=None, in_=V[:],
                        in_offset=bass.IndirectOffsetOnAxis(ap=gidx8[:, j:j + 1], axis=0))

            def stage_dc(t):
                # ---- Dc: weighted accumulate + store ----
                gat = gat_t[t]
                w8 = w8_t[t]
                acc = apool.tile([P, D_MODEL], f32, tag="acc")
                nc.vector.tensor_scalar(out=acc[:], in0=gat[:, 0:D_MODEL], scalar1=w8[:, 0:1],
                                        scalar2=None, op0=mybir.AluOpType.mult)
                for j in range(1, K_TOP):
                    nc.vector.scalar_tensor_tensor(
                        out=acc[:], in0=gat[:, j * D_MODEL:(j + 1) * D_MODEL], scalar=w8[:, j:j + 1],
                        in1=acc[:], op0=mybir.AluOpType.mult, op1=mybir.AluOpType.add)
                nc.sync.dma_start(out=out[t * P:(t + 1) * P, :], in_=acc[:])

            # software pipeline: tile t+1's B (Wg stream + PE) and tile t's
            # gathers all run during tile t's C Vector work
            stage_ab(0)
            stage_c(0); stage_ab(1); stage_da(0)
            stage_c(1); stage_ab(2); stage_da(1); stage_db(0)
            stage_c(2); stage_ab(3); stage_da(2); stage_dc(0); stage_db(1)
            stage_c(3); stage_da(3); stage_dc(1); stage_db(2)
            stage_dc(2); stage_db(3); stage_dc(3)

    nc.compile()
    return nc


_NC_CACHE = {}


def _get_nc():
    if "v4" not in _NC_CACHE:
        _NC_CACHE["v4"] = _build()
    return _NC_CACHE["v4"]


def _split16(a):
    hi = a.astype(np.float16)
    lo = (a - hi.astype(np.float32)).astype(np.float16)
    return hi, lo


def _prep_in_maps(x, router_w, compress_neurons, knowledge_K, knowledge_V):
    x = np.asarray(x, dtype=np.float32).reshape(B * S, D_MODEL) * XS
    rwT = np.ascontiguousarray(np.asarray(router_w, dtype=np.float32).T) * RWS
    rw_r = np.ascontiguousarray(
        rwT.reshape(N_DC, P, N_COMPRESS).transpose(1, 0, 2).reshape(P, N_DC * N_COMPRESS))
    rwh, rwl = _split16(rw_r)
    cn = np.asarray(compress_neurons, dtype=np.float32) * WS
    Wg1 = np.ascontiguousarray(
        cn.reshape(N_G, 4, N_DC, P, RANK).transpose(0, 2, 3, 1, 4).reshape(N_G * N_DC * P, 4 * RANK))
    Wg = np.ascontiguousarray(np.tile(Wg1, (N_TILES, 1)))   # tile-major replication
    Wgh, Wgl = _split16(Wg)
    K = np.asarray(knowledge_K, dtype=np.float32)
    KT = np.ascontiguousarray(K.T)                           # [128, 32768]
    Kh = KT.astype(np.float16)
    Vf = np.ascontiguousarray(np.asarray(knowledge_V, dtype=np.float32))
    ident = np.eye(P, dtype=np.float32)
    iota = np.tile((np.arange(SUP, dtype=np.float64) * (2.0 ** -26)).astype(np.float32), (P, 1))
    base = np.tile(((np.arange(NCAND) // 8) * SUP).astype(np.float32), (P, 1))
    Krows_f = np.ascontiguousarray(K)

    in_maps = []
    for c in range(N_CORES):
        xs = x[c * TOK_PER_CORE:(c + 1) * TOK_PER_CORE]
        xT = np.ascontiguousarray(
            xs.T.reshape(N_DC, P, TOK_PER_CORE).transpose(1, 0, 2).reshape(P, N_DC * TOK_PER_CORE))
        xhc, xlc = _split16(xT)
        in_maps.append(dict(xh=xhc, xl=xlc, rwh=rwh, rwl=rwl, Wgh=Wgh, Wgl=Wgl,
                            Kh=Kh, Krows=Krows_f, V=Vf, iotaS=iota, baseS=base,
                            ident=ident))
    return in_maps


def _ensure_ntff_hook():
    import sys as _sys
    import types as _types
    if "antenv.axon_hooks" in _sys.modules:
        return
    try:
        import antenv.axon_hooks  # noqa: F401
        return
    except ImportError:
        pass
    mod = _types.ModuleType("antenv.axon_hooks")
    _state = {"hook": None}
    mod.set_axon_ntff_profile_hook = lambda h: _state.__setitem__("hook", h)
    mod.get_axon_ntff_profile_hook = lambda: _state["hook"]
    _sys.modules["antenv.axon_hooks"] = mod
    try:
        from trn_agent_boot.trn_boot import _ntff_profile_via_ctypes
        mod.set_axon_ntff_profile_hook(_ntff_profile_via_ctypes("/opt/axon/libaxon_pjrt.so"))
    except Exception:
        pass


def _run(inputs, trace=False, dbg=False):
    if trace:
        _ensure_ntff_hook()
    nc = _get_nc()
    in_maps = _prep_in_maps(**inputs)
    res = run_bass_kernel_spmd(nc, in_maps, core_ids=list(range(N_CORES)), trace=trace)
    out = np.concatenate([res.results[c]["out"] for c in range(N_CORES)], axis=0)
    return out.reshape(B, S, D_MODEL), res


def kernel(x, router_w, compress_neurons, knowledge_K, knowledge_V):
    out, _ = _run(dict(x=x, router_w=router_w, compress_neurons=compress_neurons,
                       knowledge_K=knowledge_K, knowledge_V=knowledge_V))
    return out


# revision 21
# speedup vs baseline: 1.0561x; 1.0066x over previous
"""NeuronMemory retrieval kernel for 8 TRN2 NeuronCores — v4.

Per token: softmax-routed low-rank projection Q (rank 128), dense scores
against 32768 knowledge keys, top-8, softmax, weighted gather of V rows.
Sharding: data-parallel over the 4096 tokens (512/core); tables replicated.

Architecture (per core, 4 token tiles of 128):
  A. router scores (fp16 2-term split, prescaled) + softmax -> wts
  B. Q' = 128*Q via fp16 3-term-split matmuls + fused weighting (exact to
     ~2^-22); Wg streamed per tile so tile t+1's B overlaps tile t's C.
  T. PE-transpose -> QT (true scale SCALE*Q) -> QTh = fp16(QT) for screening
  C. screening scores s ~= QTh.T @ Kh (fp16 1-term, resident Kh) -> PSUM
     per 1024-chunk; Scalar copies +768 (rounds mantissa to 2^-14 grid);
     Vector packs p = (s768-768) + idx*2^-26 (chunk-local index embedded in
     low mantissa bits); Vector max8 per 2048 super-chunk -> 8 packed
     candidates each (16 super-chunks -> 128 candidates, provably contains
     the true top-8 up to fp16 screen noise ~3e-4 rel, covered by margin).
  D. merge: max8 + match_replace + max8 -> top-12 packed finalists; unpack
     value+index; resolve super-chunk base via is_equal one-hot dot with a
     per-slot base table.
  E. exact rescore: indirect-DMA gather the 12 candidate K rows (fp32) and
     dot with exact Q' on GpSimd -> s12 (true scale, exact to ~2^-22).
  F. top-8 of s12 + softmax + indirect-DMA gather of V rows (4KB) + fused
     weighted accumulate -> out.
"""
import numpy as np

import concourse.bacc as bacc
import concourse.bass as bass
import concourse.mybir as mybir
from concourse.tile import TileContext
from concourse.bass_utils import run_bass_kernel_spmd

P = 128
D_MODEL = 1024
RANK = 128
N_COMPRESS = 16
N_KNOWLEDGE = 32768
K_TOP = 8
B, S = 2, 2048
N_CORES = 8
TOK_PER_CORE = (B * S) // N_CORES      # 512
N_TILES = TOK_PER_CORE // P            # 4
N_DC = D_MODEL // P                    # 8
N_G = 4                                # neuron groups of 4
CH = 1024                              # PSUM score chunk (2 banks)
SUP = 2048                             # max8 super-chunk
N_SUP = N_KNOWLEDGE // SUP             # 16
NCAND = N_SUP * 8                      # 128 packed candidates per tile
MARGIN = 12
SCALE = 1.0 / np.sqrt(np.float32(RANK))

# host prescales for the exact fp16-split matmuls (router + Q projection)
XS = 4.0
RWS = 32.0
WS = 32.0
QT_ACT_SCALE = float(SCALE / (XS * WS))      # QT = SCALE*Q from Q' = 128*Q
RT_EXP_SCALE = float(1.0 / (XS * RWS))       # router scores' = 128*rs
RESC_SCALE = float(SCALE / (XS * WS))        # s = RESC_SCALE * sum(K * Q')

BIG = 768.0                                  # rounds |s|<0.25 to 2^-14 grid
DELTA = float(2.0 ** -26)                    # index step in packed mantissa
IDELTA = float(2.0 ** 26)

f32 = mybir.dt.float32
f16 = mybir.dt.float16
u32 = mybir.dt.uint32


def _build():
    nc = bacc.Bacc("TRN2", target_bir_lowering=False, debug=False, num_devices=N_CORES)

    xh = nc.declare_dram_parameter("xh", [P, N_DC * TOK_PER_CORE], f16, isOutput=False)
    xl = nc.declare_dram_parameter("xl", [P, N_DC * TOK_PER_CORE], f16, isOutput=False)
    rwh = nc.declare_dram_parameter("rwh", [P, N_DC * N_COMPRESS], f16, isOutput=False)
    rwl = nc.declare_dram_parameter("rwl", [P, N_DC * N_COMPRESS], f16, isOutput=False)
    Wgh = nc.declare_dram_parameter("Wgh", [N_TILES * N_G * N_DC * P, 512], f16, isOutput=False)
    Wgl = nc.declare_dram_parameter("Wgl", [N_TILES * N_G * N_DC * P, 512], f16, isOutput=False)
    Kh = nc.declare_dram_parameter("Kh", [P, N_KNOWLEDGE], f16, isOutput=False)
    Krows = nc.declare_dram_parameter("Krows", [N_KNOWLEDGE, RANK], f32, isOutput=False)
    V = nc.declare_dram_parameter("V", [N_KNOWLEDGE, D_MODEL], f32, isOutput=False)
    iotaS = nc.declare_dram_parameter("iotaS", [P, SUP], f32, isOutput=False)
    baseS = nc.declare_dram_parameter("baseS", [P, NCAND], f32, isOutput=False)
    ident = nc.declare_dram_parameter("ident", [P, P], f32, isOutput=False)
    out = nc.declare_dram_parameter("out", [TOK_PER_CORE, D_MODEL], f32, isOutput=True)

    # Wg is replicated once per tile so B can stream tile-major:
    Wgh_v = Wgh.rearrange("(t g dc p) n -> t g dc p n", t=N_TILES, g=N_G, dc=N_DC)
    Wgl_v = Wgl.rearrange("(t g dc p) n -> t g dc p n", t=N_TILES, g=N_G, dc=N_DC)

    with TileContext(nc) as tc:
        with (
            tc.tile_pool(name="const", bufs=1) as cpool,
            tc.tile_pool(name="wld", bufs=6) as wpool,
            tc.tile_pool(name="s768", bufs=2) as spool768,
            tc.tile_pool(name="pack", bufs=2) as ppool,
            tc.tile_pool(name="cand", bufs=2) as candpool,
            tc.tile_pool(name="kg", bufs=2) as kgpool,
            tc.tile_pool(name="gat", bufs=1) as gpool,
            tc.tile_pool(name="acc", bufs=2) as apool,
            tc.tile_pool(name="small", bufs=6) as spool,
            tc.tile_pool(name="ps_sc", bufs=2, space="PSUM") as ps_sc,
            tc.tile_pool(name="ps_y", bufs=2, space="PSUM") as ps_y,
            tc.tile_pool(name="ps_sm", bufs=1, space="PSUM") as ps_sm,
        ):
            # ---- persistent loads ----
            xh_sb = cpool.tile([P, N_DC * TOK_PER_CORE], f16)
            xl_sb = cpool.tile([P, N_DC * TOK_PER_CORE], f16)
            rwh_sb = cpool.tile([P, N_DC * N_COMPRESS], f16)
            rwl_sb = cpool.tile([P, N_DC * N_COMPRESS], f16)
            id_sb = cpool.tile([P, P], f32)
            kh_sb = cpool.tile([P, N_KNOWLEDGE], f16)       # 32KB/part resident
            iota_sb = cpool.tile([P, SUP], f32)
            base_sb = cpool.tile([P, NCAND], f32)
            nc.sync.dma_start(out=xh_sb[:], in_=xh[:])
            nc.sync.dma_start(out=xl_sb[:], in_=xl[:])
            nc.sync.dma_start(out=rwh_sb[:], in_=rwh[:])
            nc.sync.dma_start(out=rwl_sb[:], in_=rwl[:])
            nc.sync.dma_start(out=id_sb[:], in_=ident[:])

            wts_sb = cpool.tile([P, N_TILES * N_COMPRESS], f32)
            Q_sb = cpool.tile([P, N_TILES * RANK], f32)      # Q' = 128*Q, exact
            QT_sb = cpool.tile([P, N_TILES * P], f32)        # SCALE*Q
            QTh_sb = cpool.tile([P, N_TILES * P], f16)       # screen lhsT

            def tok(t):
                return slice(t * P, (t + 1) * P)

            cand_t = {}
            gixf_t = {}
            gix12_t = {}
            kg_t = {}
            w8_t = {}
            gidx8_t = {}
            gat_t = {}

            def stage_ab(t):
                # ---- A: router softmax (fp16 2-term, exact) ----
                rps = ps_sm.tile([P, N_COMPRESS], f32, space="PSUM", tag="rps")
                n_mm = N_DC * 3
                i_mm = 0
                for dc in range(N_DC):
                    xsl = slice(dc * TOK_PER_CORE + t * P, dc * TOK_PER_CORE + (t + 1) * P)
                    rsl = slice(dc * N_COMPRESS, (dc + 1) * N_COMPRESS)
                    for lhs, rhs in ((xh_sb, rwh_sb), (xh_sb, rwl_sb), (xl_sb, rwh_sb)):
                        nc.tensor.matmul(out=rps[:], lhsT=lhs[:, xsl], rhs=rhs[:, rsl],
                                         start=(i_mm == 0), stop=(i_mm == n_mm - 1))
                        i_mm += 1
                w = wts_sb[:, t * N_COMPRESS:(t + 1) * N_COMPRESS]
                mx = spool.tile([P, 1], f32, tag="mx")
                sm = spool.tile([P, 1], f32, tag="sm")
                ex = spool.tile([P, N_COMPRESS], f32, tag="ex")
                nc.vector.tensor_reduce(out=mx[:], in_=rps[:], op=mybir.AluOpType.max, axis=mybir.AxisListType.X)
                nc.vector.tensor_scalar(out=ex[:], in0=rps[:], scalar1=mx[:, :1], scalar2=None, op0=mybir.AluOpType.subtract)
                nc.scalar.activation(out=ex[:], in_=ex[:], func=mybir.ActivationFunctionType.Exp,
                                     scale=RT_EXP_SCALE, accum_out=sm[:, :1])
                rcp = spool.tile([P, 1], f32, tag="rcp")
                nc.vector.reciprocal(out=rcp[:], in_=sm[:, :1])
                nc.vector.tensor_scalar(out=w, in0=ex[:], scalar1=rcp[:, :1], scalar2=None, op0=mybir.AluOpType.mult)

                # ---- B: exact Q' (fp16 3-term), Wg streamed tile-major ----
                q = Q_sb[:, t * RANK:(t + 1) * RANK]
                for g in range(N_G):
                    yps = ps_y.tile([P, 512], f32, space="PSUM", tag="yps")
                    for dc in range(N_DC):
                        wh = wpool.tile([P, 512], f16, tag="wldh")
                        wl = wpool.tile([P, 512], f16, tag="wldl")
                        nc.sync.dma_start(out=wh[:], in_=Wgh_v[t, g, dc])
                        nc.sync.dma_start(out=wl[:], in_=Wgl_v[t, g, dc])
                        xsl = slice(dc * TOK_PER_CORE + t * P, dc * TOK_PER_CORE + (t + 1) * P)
                        for j, (lhs, rhs) in enumerate(((xh_sb, wh), (xh_sb, wl), (xl_sb, wh))):
                            nc.tensor.matmul(out=yps[:], lhsT=lhs[:, xsl], rhs=rhs[:],
                                             start=(dc == 0 and j == 0),
                                             stop=(dc == N_DC - 1 and j == 2))
                    for n in range(4):
                        ncomp = g * 4 + n
                        wcol = wts_sb[:, t * N_COMPRESS + ncomp:t * N_COMPRESS + ncomp + 1]
                        ypart = yps[:, n * RANK:(n + 1) * RANK]
                        if g == 0 and n == 0:
                            nc.vector.tensor_scalar(out=q, in0=ypart, scalar1=wcol, scalar2=None,
                                                    op0=mybir.AluOpType.mult)
                        else:
                            nc.vector.scalar_tensor_tensor(out=q, in0=ypart, scalar=wcol, in1=q,
                                                           op0=mybir.AluOpType.mult,
                                                           op1=mybir.AluOpType.add)

                # ---- T: transpose -> QT (true scale), screen cast ----
                tps = ps_sm.tile([P, P], f32, space="PSUM", tag="tps")
                nc.tensor.transpose(out=tps[:], in_=Q_sb[:, t * RANK:(t + 1) * RANK], identity=id_sb[:])
                nc.scalar.activation(out=QT_sb[:, tok(t)], in_=tps[:],
                                     func=mybir.ActivationFunctionType.Copy, scale=QT_ACT_SCALE)
                nc.vector.tensor_copy(out=QTh_sb[:, tok(t)], in_=QT_sb[:, tok(t)])

            def stage_c(t):
                # ---- C: screen scores, pack, per-super-chunk max8 ----
                cand = candpool.tile([P, NCAND], f32, tag="cand")
                cand_t[t] = cand
                for sup in range(N_SUP):
                    s768 = spool768.tile([P, SUP], f32, tag="s768")
                    pk = ppool.tile([P, SUP], f32, tag="pk")
                    for h in range(2):
                        sps = ps_sc.tile([P, CH], f32, space="PSUM", tag="sc")
                        for hh in range(2):
                            ksl = slice(sup * SUP + h * CH + hh * 512,
                                        sup * SUP + h * CH + (hh + 1) * 512)
                            nc.tensor.matmul(out=sps[:, hh * 512:(hh + 1) * 512],
                                             lhsT=QTh_sb[:, tok(t)], rhs=kh_sb[:, ksl],
                                             start=True, stop=True)
                        nc.scalar.activation(out=s768[:, h * CH:(h + 1) * CH], in_=sps[:],
                                             func=mybir.ActivationFunctionType.Copy, bias=BIG)
                        nc.vector.scalar_tensor_tensor(
                            out=pk[:, h * CH:(h + 1) * CH], in0=s768[:, h * CH:(h + 1) * CH],
                            scalar=-BIG, in1=iota_sb[:, h * CH:(h + 1) * CH],
                            op0=mybir.AluOpType.add, op1=mybir.AluOpType.add)
                    nc.vector.max(out=cand[:, sup * 8:(sup + 1) * 8], in_=pk[:])

            def stage_da(t):
                # ---- Da: merge to top-12 packed, unpack, resolve, K-gather ----
                cand = cand_t[t]
                top8 = spool.tile([P, 8], f32, tag="top8")
                nxt8 = spool.tile([P, 8], f32, tag="nxt8")
                cmr = candpool.tile([P, NCAND], f32, tag="cmr")
                nc.vector.max(out=top8[:], in_=cand[:])
                nc.vector.match_replace(out=cmr[:], in_to_replace=top8[:], in_values=cand[:],
                                        imm_value=-1e30)
                nc.vector.max(out=nxt8[:], in_=cmr[:])
                pk12 = spool.tile([P, MARGIN], f32, tag="pk12")
                nc.vector.tensor_copy(out=pk12[:, 0:8], in_=top8[:])
                nc.vector.tensor_copy(out=pk12[:, 8:MARGIN], in_=nxt8[:, 0:MARGIN - 8])
                q12 = spool.tile([P, MARGIN], f32, tag="q12")
                nc.vector.tensor_scalar(out=q12[:], in0=pk12[:], scalar1=BIG, scalar2=None,
                                        op0=mybir.AluOpType.add)
                nc.vector.tensor_scalar(out=q12[:], in0=q12[:], scalar1=BIG, scalar2=None,
                                        op0=mybir.AluOpType.subtract)
                lidx = spool.tile([P, MARGIN], f32, tag="lidx")
                nc.vector.tensor_tensor(out=lidx[:], in0=pk12[:], in1=q12[:], op=mybir.AluOpType.subtract)
                nc.vector.tensor_scalar(out=lidx[:], in0=lidx[:], scalar1=IDELTA, scalar2=None,
                                        op0=mybir.AluOpType.mult)
                gixf = spool.tile([P, MARGIN], f32, tag="gixf")
                gixf_t[t] = gixf
                junk = spool.tile([P, NCAND], f32, tag="junk")
                for j in range(MARGIN):
                    nc.vector.scalar_tensor_tensor(
                        out=junk[:], in0=cand[:], scalar=pk12[:, j:j + 1], in1=base_sb[:],
                        op0=mybir.AluOpType.is_equal, op1=mybir.AluOpType.mult,
                        accum_out=gixf[:, j:j + 1])
                nc.vector.tensor_tensor(out=gixf[:], in0=gixf[:], in1=lidx[:], op=mybir.AluOpType.add)
                gix12 = spool.tile([P, MARGIN], u32, tag="gix12")
                gix12_t[t] = gix12
                nc.vector.tensor_copy(out=gix12[:], in_=gixf[:])
                kg = kgpool.tile([P, MARGIN * RANK], f32, tag="kg")
                kg_t[t] = kg
                for j in range(MARGIN):
                    nc.gpsimd.indirect_dma_start(
                        out=kg[:, j * RANK:(j + 1) * RANK], out_offset=None, in_=Krows[:],
                        in_offset=bass.IndirectOffsetOnAxis(ap=gix12[:, j:j + 1], axis=0))

            def stage_db(t):
                # ---- Db: exact rescore, top-8, softmax, V-gather ----
                kg = kg_t[t]
                gixf = gixf_t[t]
                s12 = spool.tile([P, MARGIN], f32, tag="s12")
                junk2 = spool.tile([P, RANK], f32, tag="junk2")
                for j in range(MARGIN):
                    nc.vector.scalar_tensor_tensor(
                        out=junk2[:], in0=kg[:, j * RANK:(j + 1) * RANK], scalar=RESC_SCALE,
                        in1=Q_sb[:, t * RANK:(t + 1) * RANK],
                        op0=mybir.AluOpType.mult, op1=mybir.AluOpType.mult,
                        accum_out=s12[:, j:j + 1])
                v8 = spool.tile([P, 8], f32, tag="v8")
                nc.vector.max(out=v8[:], in_=s12[:])
                gf8 = spool.tile([P, 8], f32, tag="gf8")
                junk3 = spool.tile([P, MARGIN], f32, tag="junk3")
                for j in range(K_TOP):
                    nc.vector.scalar_tensor_tensor(
                        out=junk3[:], in0=s12[:], scalar=v8[:, j:j + 1], in1=gixf[:],
                        op0=mybir.AluOpType.is_equal, op1=mybir.AluOpType.mult,
                        accum_out=gf8[:, j:j + 1])
                gidx8 = spool.tile([P, 8], u32, tag="gidx8")
                gidx8_t[t] = gidx8
                nc.vector.tensor_copy(out=gidx8[:], in_=gf8[:])

                w8 = spool.tile([P, 8], f32, tag="w8")
                w8_t[t] = w8
                sm8 = spool.tile([P, 1], f32, tag="sm8")
                nc.vector.tensor_scalar(out=w8[:], in0=v8[:], scalar1=v8[:, :1], scalar2=None,
                                        op0=mybir.AluOpType.subtract)
                nc.scalar.activation(out=w8[:], in_=w8[:], func=mybir.ActivationFunctionType.Exp,
                                     accum_out=sm8[:, :1])
                rcp8 = spool.tile([P, 1], f32, tag="rcp8")
                nc.vector.reciprocal(out=rcp8[:], in_=sm8[:, :1])
                nc.vector.tensor_scalar(out=w8[:], in0=w8[:], scalar1=rcp8[:, :1], scalar2=None,
                                        op0=mybir.AluOpType.mult)

                gat = gpool.tile([P, K_TOP * D_MODEL], f32, tag="gat")
                gat_t[t] = gat
                for j in range(K_TOP):
                    nc.gpsimd.indirect_dma_start(
                        out=gat[:, j * D_MODEL:(j + 1) * D_MODEL], out_offset=None, in_=V[:],
                        in_offset=bass.IndirectOffsetOnAxis(ap=gidx8[:, j:j + 1], axis=0))

            def stage_dc(t):
                # ---- Dc: weighted accumulate + store ----
                gat = gat_t[t]
                w8 = w8_t[t]
                acc = apool.tile([P, D_MODEL], f32, tag="acc")
                nc.vector.tensor_scalar(out=acc[:], in0=gat[:, 0:D_MODEL], scalar1=w8[:, 0:1],
                                        scalar2=None, op0=mybir.AluOpType.mult)
                for j in range(1, K_TOP):
                    nc.vector.scalar_tensor_tensor(
                        out=acc[:], in0=gat[:, j * D_MODEL:(j + 1) * D_MODEL], scalar=w8[:, j:j + 1],
                        in1=acc[:], op0=mybir.AluOpType.mult, op1=mybir.AluOpType.add)
                nc.sync.dma_start(out=out[t * P:(t + 1) * P, :], in_=acc[:])

            nc.gpsimd.dma_start(out=kh_sb[:], in_=Kh[:])
            nc.gpsimd.dma_start(out=iota_sb[:], in_=iotaS[:])
            nc.gpsimd.dma_start(out=base_sb[:], in_=baseS[:])

            # software pipeline: tile t+1's B (Wg stream + PE) and tile t's
            # gathers all run during tile t's C Vector work
            stage_ab(0)
            stage_c(0); stage_ab(1); stage_da(0)
            stage_c(1); stage_ab(2); stage_da(1); stage_db(0)
            stage_c(2); stage_ab(3); stage_da(2); stage_dc(0); stage_db(1)
            stage_c(3); stage_da(3); stage_dc(1); stage_db(2)
            stage_dc(2); stage_db(3); stage_dc(3)

    nc.compile()
    return nc


_NC_CACHE = {}


def _get_nc():
    if "v4" not in _NC_CACHE:
        _NC_CACHE["v4"] = _build()
    return _NC_CACHE["v4"]


def _split16(a):
    hi = a.astype(np.float16)
    lo = (a - hi.astype(np.float32)).astype(np.float16)
    return hi, lo


def _prep_in_maps(x, router_w, compress_neurons, knowledge_K, knowledge_V):
    x = np.asarray(x, dtype=np.float32).reshape(B * S, D_MODEL) * XS
    rwT = np.ascontiguousarray(np.asarray(router_w, dtype=np.float32).T) * RWS
    rw_r = np.ascontiguousarray(
        rwT.reshape(N_DC, P, N_COMPRESS).transpose(1, 0, 2).reshape(P, N_DC * N_COMPRESS))
    rwh, rwl = _split16(rw_r)
    cn = np.asarray(compress_neurons, dtype=np.float32) * WS
    Wg1 = np.ascontiguousarray(
        cn.reshape(N_G, 4, N_DC, P, RANK).transpose(0, 2, 3, 1, 4).reshape(N_G * N_DC * P, 4 * RANK))
    Wg = np.ascontiguousarray(np.tile(Wg1, (N_TILES, 1)))   # tile-major replication
    Wgh, Wgl = _split16(Wg)
    K = np.asarray(knowledge_K, dtype=np.float32)
    KT = np.ascontiguousarray(K.T)                           # [128, 32768]
    Kh = KT.astype(np.float16)
    Vf = np.ascontiguousarray(np.asarray(knowledge_V, dtype=np.float32))
    ident = np.eye(P, dtype=np.float32)
    iota = np.tile((np.arange(SUP, dtype=np.float64) * (2.0 ** -26)).astype(np.float32), (P, 1))
    base = np.tile(((np.arange(NCAND) // 8) * SUP).astype(np.float32), (P, 1))
    Krows_f = np.ascontiguousarray(K)

    in_maps = []
    for c in range(N_CORES):
        xs = x[c * TOK_PER_CORE:(c + 1) * TOK_PER_CORE]
        xT = np.ascontiguousarray(
            xs.T.reshape(N_DC, P, TOK_PER_CORE).transpose(1, 0, 2).reshape(P, N_DC * TOK_PER_CORE))
        xhc, xlc = _split16(xT)
        in_maps.append(dict(xh=xhc, xl=xlc, rwh=rwh, rwl=rwl, Wgh=Wgh, Wgl=Wgl,
                            Kh=Kh, Krows=Krows_f, V=Vf, iotaS=iota, baseS=base,
                            ident=ident))
    return in_maps


def _ensure_ntff_hook():
    import sys as _sys
    import types as _types
    if "antenv.axon_hooks" in _sys.modules:
        return
    try:
        import antenv.axon_hooks  # noqa: F401
        return
    except ImportError:
        pass
    mod = _types.ModuleType("antenv.axon_hooks")
    _state = {"hook": None}
    mod.set_axon_ntff_profile_hook = lambda h: _state.__setitem__("hook", h)
    mod.get_axon_ntff_profile_hook = lambda: _state["hook"]
    _sys.modules["antenv.axon_hooks"] = mod
    try:
        from trn_agent_boot.trn_boot import _ntff_profile_via_ctypes
        mod.set_axon_ntff_profile_hook(_ntff_profile_via_ctypes("/opt/axon/libaxon_pjrt.so"))
    except Exception:
        pass


def _run(inputs, trace=False, dbg=False):
    if trace:
        _ensure_ntff_hook()
    nc = _get_nc()
    in_maps = _prep_in_maps(**inputs)
    res = run_bass_kernel_spmd(nc, in_maps, core_ids=list(range(N_CORES)), trace=trace)
    out = np.concatenate([res.results[c]["out"] for c in range(N_CORES)], axis=0)
    return out.reshape(B, S, D_MODEL), res


def kernel(x, router_w, compress_neurons, knowledge_K, knowledge_V):
    out, _ = _run(dict(x=x, router_w=router_w, compress_neurons=compress_neurons,
                       knowledge_K=knowledge_K, knowledge_V=knowledge_V))
    return out


# revision 22
# speedup vs baseline: 1.1219x; 1.0624x over previous
"""NeuronMemory retrieval kernel for 8 TRN2 NeuronCores — v4.

Per token: softmax-routed low-rank projection Q (rank 128), dense scores
against 32768 knowledge keys, top-8, softmax, weighted gather of V rows.
Sharding: data-parallel over the 4096 tokens (512/core); tables replicated.

Architecture (per core, 4 token tiles of 128):
  A. router scores (fp16 2-term split, prescaled) + softmax -> wts
  B. Q' = 128*Q via fp16 3-term-split matmuls + fused weighting (exact to
     ~2^-22); Wg streamed per tile so tile t+1's B overlaps tile t's C.
  T. PE-transpose -> QT (true scale SCALE*Q) -> QTh = fp16(QT) for screening
  C. screening scores s ~= QTh.T @ Kh (fp16 1-term, resident Kh) -> PSUM
     per 1024-chunk; Scalar copies +768 (rounds mantissa to 2^-14 grid);
     Vector packs p = (s768-768) + idx*2^-26 (chunk-local index embedded in
     low mantissa bits); Vector max8 per 2048 super-chunk -> 8 packed
     candidates each (16 super-chunks -> 128 candidates, provably contains
     the true top-8 up to fp16 screen noise ~3e-4 rel, covered by margin).
  D. merge: max8 + match_replace + max8 -> top-12 packed finalists; unpack
     value+index; resolve super-chunk base via is_equal one-hot dot with a
     per-slot base table.
  E. exact rescore: indirect-DMA gather the 12 candidate K rows (fp32) and
     dot with exact Q' on GpSimd -> s12 (true scale, exact to ~2^-22).
  F. top-8 of s12 + softmax + indirect-DMA gather of V rows (4KB) + fused
     weighted accumulate -> out.
"""
import numpy as np

import concourse.bacc as bacc
import concourse.bass as bass
import concourse.mybir as mybir
from concourse.tile import TileContext
from concourse.bass_utils import run_bass_kernel_spmd

P = 128
D_MODEL = 1024
RANK = 128
N_COMPRESS = 16
N_KNOWLEDGE = 32768
K_TOP = 8
B, S = 2, 2048
N_CORES = 8
TOK_PER_CORE = (B * S) // N_CORES      # 512
N_TILES = TOK_PER_CORE // P            # 4
N_DC = D_MODEL // P                    # 8
N_G = 4                                # neuron groups of 4
CH = 1024                              # PSUM score chunk (2 banks)
SUP = 2048                             # max8 super-chunk
N_SUP = N_KNOWLEDGE // SUP             # 16
NCAND = N_SUP * 8                      # 128 packed candidates per tile
MARGIN = 12
SCALE = 1.0 / np.sqrt(np.float32(RANK))

# host prescales for the exact fp16-split matmuls (router + Q projection)
XS = 4.0
RWS = 32.0
WS = 32.0
QT_ACT_SCALE = float(SCALE / (XS * WS))      # QT = SCALE*Q from Q' = 128*Q
RT_EXP_SCALE = float(1.0 / (XS * RWS))       # router scores' = 128*rs
RESC_SCALE = float(SCALE / (XS * WS))        # s = RESC_SCALE * sum(K * Q')

BIG = 768.0                                  # rounds |s|<0.25 to 2^-14 grid
DELTA = float(2.0 ** -26)                    # index step in packed mantissa
IDELTA = float(2.0 ** 26)

f32 = mybir.dt.float32
f16 = mybir.dt.float16
u32 = mybir.dt.uint32


def _build():
    nc = bacc.Bacc("TRN2", target_bir_lowering=False, debug=False, num_devices=N_CORES)

    xh = nc.declare_dram_parameter("xh", [P, N_DC * TOK_PER_CORE], f16, isOutput=False)
    xl = nc.declare_dram_parameter("xl", [P, N_DC * TOK_PER_CORE], f16, isOutput=False)
    rwh = nc.declare_dram_parameter("rwh", [P, N_DC * N_COMPRESS], f16, isOutput=False)
    rwl = nc.declare_dram_parameter("rwl", [P, N_DC * N_COMPRESS], f16, isOutput=False)
    Wgh = nc.declare_dram_parameter("Wgh", [N_TILES * N_G * N_DC * P, 512], f16, isOutput=False)
    Wgl = nc.declare_dram_parameter("Wgl", [N_TILES * N_G * N_DC * P, 512], f16, isOutput=False)
    Kh = nc.declare_dram_parameter("Kh", [P, N_KNOWLEDGE], f16, isOutput=False)
    Krows = nc.declare_dram_parameter("Krows", [N_KNOWLEDGE, RANK], f32, isOutput=False)
    V = nc.declare_dram_parameter("V", [N_KNOWLEDGE, D_MODEL], f32, isOutput=False)
    iotaS = nc.declare_dram_parameter("iotaS", [P, SUP], f32, isOutput=False)
    baseS = nc.declare_dram_parameter("baseS", [P, NCAND], f32, isOutput=False)
    ident = nc.declare_dram_parameter("ident", [P, P], f32, isOutput=False)
    out = nc.declare_dram_parameter("out", [TOK_PER_CORE, D_MODEL], f32, isOutput=True)

    # Wg is replicated once per tile so B can stream tile-major:
    Wgh_v = Wgh.rearrange("(t g dc p) n -> t g dc p n", t=N_TILES, g=N_G, dc=N_DC)
    Wgl_v = Wgl.rearrange("(t g dc p) n -> t g dc p n", t=N_TILES, g=N_G, dc=N_DC)

    with TileContext(nc) as tc:
        with (
            tc.tile_pool(name="const", bufs=1) as cpool,
            tc.tile_pool(name="wld", bufs=6) as wpool,
            tc.tile_pool(name="s768", bufs=2) as spool768,
            tc.tile_pool(name="pack", bufs=2) as ppool,
            tc.tile_pool(name="cand", bufs=2) as candpool,
            tc.tile_pool(name="kg", bufs=2) as kgpool,
            tc.tile_pool(name="gat", bufs=1) as gpool,
            tc.tile_pool(name="acc", bufs=2) as apool,
            tc.tile_pool(name="small", bufs=6) as spool,
            tc.tile_pool(name="ps_sc", bufs=2, space="PSUM") as ps_sc,
            tc.tile_pool(name="ps_y", bufs=2, space="PSUM") as ps_y,
            tc.tile_pool(name="ps_sm", bufs=1, space="PSUM") as ps_sm,
        ):
            # ---- persistent loads ----
            xh_sb = cpool.tile([P, N_DC * TOK_PER_CORE], f16)
            xl_sb = cpool.tile([P, N_DC * TOK_PER_CORE], f16)
            rwh_sb = cpool.tile([P, N_DC * N_COMPRESS], f16)
            rwl_sb = cpool.tile([P, N_DC * N_COMPRESS], f16)
            id_sb = cpool.tile([P, P], f32)
            kh_sb = cpool.tile([P, N_KNOWLEDGE], f16)       # 32KB/part resident
            iota_sb = cpool.tile([P, SUP], f32)
            base_sb = cpool.tile([P, NCAND], f32)
            nc.sync.dma_start(out=xh_sb[:], in_=xh[:])
            nc.sync.dma_start(out=xl_sb[:], in_=xl[:])
            nc.sync.dma_start(out=rwh_sb[:], in_=rwh[:])
            nc.sync.dma_start(out=rwl_sb[:], in_=rwl[:])
            nc.sync.dma_start(out=id_sb[:], in_=ident[:])

            wts_sb = cpool.tile([P, N_TILES * N_COMPRESS], f32)
            Q_sb = cpool.tile([P, N_TILES * RANK], f32)      # Q' = 128*Q, exact
            QT_sb = cpool.tile([P, N_TILES * P], f32)        # SCALE*Q
            QTh_sb = cpool.tile([P, N_TILES * P], f16)       # screen lhsT

            def tok(t):
                return slice(t * P, (t + 1) * P)

            cand_t = {}
            gixf_t = {}
            gix12_t = {}
            kg_t = {}
            w8_t = {}
            gidx8_t = {}
            gat_t = {}

            def stage_ab(t):
                # ---- A: router softmax (fp16 2-term, exact) ----
                rps = ps_sm.tile([P, N_COMPRESS], f32, space="PSUM", tag="rps")
                n_mm = N_DC * 3
                i_mm = 0
                for dc in range(N_DC):
                    xsl = slice(dc * TOK_PER_CORE + t * P, dc * TOK_PER_CORE + (t + 1) * P)
                    rsl = slice(dc * N_COMPRESS, (dc + 1) * N_COMPRESS)
                    for lhs, rhs in ((xh_sb, rwh_sb), (xh_sb, rwl_sb), (xl_sb, rwh_sb)):
                        nc.tensor.matmul(out=rps[:], lhsT=lhs[:, xsl], rhs=rhs[:, rsl],
                                         start=(i_mm == 0), stop=(i_mm == n_mm - 1))
                        i_mm += 1
                w = wts_sb[:, t * N_COMPRESS:(t + 1) * N_COMPRESS]
                mx = spool.tile([P, 1], f32, tag="mx")
                sm = spool.tile([P, 1], f32, tag="sm")
                ex = spool.tile([P, N_COMPRESS], f32, tag="ex")
                nc.vector.tensor_reduce(out=mx[:], in_=rps[:], op=mybir.AluOpType.max, axis=mybir.AxisListType.X)
                nc.vector.tensor_scalar(out=ex[:], in0=rps[:], scalar1=mx[:, :1], scalar2=None, op0=mybir.AluOpType.subtract)
                nc.scalar.activation(out=ex[:], in_=ex[:], func=mybir.ActivationFunctionType.Exp,
                                     scale=RT_EXP_SCALE, accum_out=sm[:, :1])
                rcp = spool.tile([P, 1], f32, tag="rcp")
                nc.vector.reciprocal(out=rcp[:], in_=sm[:, :1])
                nc.vector.tensor_scalar(out=w, in0=ex[:], scalar1=rcp[:, :1], scalar2=None, op0=mybir.AluOpType.mult)

                # ---- B: exact Q' (fp16 3-term), Wg streamed tile-major ----
                q = Q_sb[:, t * RANK:(t + 1) * RANK]
                for g in range(N_G):
                    yps = ps_y.tile([P, 512], f32, space="PSUM", tag="yps")
                    for dc in range(N_DC):
                        wh = wpool.tile([P, 512], f16, tag="wldh")
                        wl = wpool.tile([P, 512], f16, tag="wldl")
                        nc.sync.dma_start(out=wh[:], in_=Wgh_v[t, g, dc])
                        nc.sync.dma_start(out=wl[:], in_=Wgl_v[t, g, dc])
                        xsl = slice(dc * TOK_PER_CORE + t * P, dc * TOK_PER_CORE + (t + 1) * P)
                        for j, (lhs, rhs) in enumerate(((xh_sb, wh), (xh_sb, wl), (xl_sb, wh))):
                            nc.tensor.matmul(out=yps[:], lhsT=lhs[:, xsl], rhs=rhs[:],
                                             start=(dc == 0 and j == 0),
                                             stop=(dc == N_DC - 1 and j == 2))
                    for n in range(4):
                        ncomp = g * 4 + n
                        wcol = wts_sb[:, t * N_COMPRESS + ncomp:t * N_COMPRESS + ncomp + 1]
                        ypart = yps[:, n * RANK:(n + 1) * RANK]
                        if g == 0 and n == 0:
                            nc.vector.tensor_scalar(out=q, in0=ypart, scalar1=wcol, scalar2=None,
                                                    op0=mybir.AluOpType.mult)
                        else:
                            nc.vector.scalar_tensor_tensor(out=q, in0=ypart, scalar=wcol, in1=q,
                                                           op0=mybir.AluOpType.mult,
                                                           op1=mybir.AluOpType.add)

                # ---- T: transpose -> QT (true scale), screen cast ----
                tps = ps_sm.tile([P, P], f32, space="PSUM", tag="tps")
                nc.tensor.transpose(out=tps[:], in_=Q_sb[:, t * RANK:(t + 1) * RANK], identity=id_sb[:])
                nc.scalar.activation(out=QT_sb[:, tok(t)], in_=tps[:],
                                     func=mybir.ActivationFunctionType.Copy, scale=QT_ACT_SCALE)
                nc.vector.tensor_copy(out=QTh_sb[:, tok(t)], in_=QT_sb[:, tok(t)])

            def stage_c(t):
                # ---- C: screen scores, pack, per-super-chunk max8 ----
                cand = candpool.tile([P, NCAND], f32, tag="cand")
                cand_t[t] = cand
                for sup in range(N_SUP):
                    s768 = spool768.tile([P, SUP], f32, tag="s768")
                    pk = ppool.tile([P, SUP], f32, tag="pk")
                    for h in range(2):
                        sps = ps_sc.tile([P, CH], f32, space="PSUM", tag="sc")
                        for hh in range(2):
                            ksl = slice(sup * SUP + h * CH + hh * 512,
                                        sup * SUP + h * CH + (hh + 1) * 512)
                            nc.tensor.matmul(out=sps[:, hh * 512:(hh + 1) * 512],
                                             lhsT=QTh_sb[:, tok(t)], rhs=kh_sb[:, ksl],
                                             start=True, stop=True)
                        nc.scalar.activation(out=s768[:, h * CH:(h + 1) * CH], in_=sps[:],
                                             func=mybir.ActivationFunctionType.Copy, bias=BIG)
                        nc.scalar.activation(out=pk[:, h * CH:(h + 1) * CH],
                                             in_=s768[:, h * CH:(h + 1) * CH],
                                             func=mybir.ActivationFunctionType.Copy, bias=-BIG)
                        nc.vector.tensor_tensor(
                            out=pk[:, h * CH:(h + 1) * CH], in0=pk[:, h * CH:(h + 1) * CH],
                            in1=iota_sb[:, h * CH:(h + 1) * CH], op=mybir.AluOpType.add)
                    nc.vector.max(out=cand[:, sup * 8:(sup + 1) * 8], in_=pk[:])

            def stage_da(t):
                # ---- Da: merge to top-12 packed, unpack, resolve, K-gather ----
                cand = cand_t[t]
                top8 = spool.tile([P, 8], f32, tag="top8")
                nxt8 = spool.tile([P, 8], f32, tag="nxt8")
                cmr = candpool.tile([P, NCAND], f32, tag="cmr")
                nc.vector.max(out=top8[:], in_=cand[:])
                nc.vector.match_replace(out=cmr[:], in_to_replace=top8[:], in_values=cand[:],
                                        imm_value=-1e30)
                nc.vector.max(out=nxt8[:], in_=cmr[:])
                pk12 = spool.tile([P, MARGIN], f32, tag="pk12")
                nc.vector.tensor_copy(out=pk12[:, 0:8], in_=top8[:])
                nc.vector.tensor_copy(out=pk12[:, 8:MARGIN], in_=nxt8[:, 0:MARGIN - 8])
                q12 = spool.tile([P, MARGIN], f32, tag="q12")
                nc.vector.tensor_scalar(out=q12[:], in0=pk12[:], scalar1=BIG, scalar2=None,
                                        op0=mybir.AluOpType.add)
                nc.vector.tensor_scalar(out=q12[:], in0=q12[:], scalar1=BIG, scalar2=None,
                                        op0=mybir.AluOpType.subtract)
                lidx = spool.tile([P, MARGIN], f32, tag="lidx")
                nc.vector.tensor_tensor(out=lidx[:], in0=pk12[:], in1=q12[:], op=mybir.AluOpType.subtract)
                nc.vector.tensor_scalar(out=lidx[:], in0=lidx[:], scalar1=IDELTA, scalar2=None,
                                        op0=mybir.AluOpType.mult)
                gixf = spool.tile([P, MARGIN], f32, tag="gixf")
                gixf_t[t] = gixf
                junk = spool.tile([P, NCAND], f32, tag="junk")
                for j in range(MARGIN):
                    nc.vector.scalar_tensor_tensor(
                        out=junk[:], in0=cand[:], scalar=pk12[:, j:j + 1], in1=base_sb[:],
                        op0=mybir.AluOpType.is_equal, op1=mybir.AluOpType.mult,
                        accum_out=gixf[:, j:j + 1])
                nc.vector.tensor_tensor(out=gixf[:], in0=gixf[:], in1=lidx[:], op=mybir.AluOpType.add)
                gix12 = spool.tile([P, MARGIN], u32, tag="gix12")
                gix12_t[t] = gix12
                nc.vector.tensor_copy(out=gix12[:], in_=gixf[:])
                kg = kgpool.tile([P, MARGIN * RANK], f32, tag="kg")
                kg_t[t] = kg
                for j in range(MARGIN):
                    nc.gpsimd.indirect_dma_start(
                        out=kg[:, j * RANK:(j + 1) * RANK], out_offset=None, in_=Krows[:],
                        in_offset=bass.IndirectOffsetOnAxis(ap=gix12[:, j:j + 1], axis=0))

            def stage_db(t):
                # ---- Db: exact rescore, top-8, softmax, V-gather ----
                kg = kg_t[t]
                gixf = gixf_t[t]
                s12 = spool.tile([P, MARGIN], f32, tag="s12")
                junk2 = spool.tile([P, RANK], f32, tag="junk2")
                for j in range(MARGIN):
                    nc.vector.scalar_tensor_tensor(
                        out=junk2[:], in0=kg[:, j * RANK:(j + 1) * RANK], scalar=RESC_SCALE,
                        in1=Q_sb[:, t * RANK:(t + 1) * RANK],
                        op0=mybir.AluOpType.mult, op1=mybir.AluOpType.mult,
                        accum_out=s12[:, j:j + 1])
                v8 = spool.tile([P, 8], f32, tag="v8")
                nc.vector.max(out=v8[:], in_=s12[:])
                gf8 = spool.tile([P, 8], f32, tag="gf8")
                junk3 = spool.tile([P, MARGIN], f32, tag="junk3")
                for j in range(K_TOP):
                    nc.vector.scalar_tensor_tensor(
                        out=junk3[:], in0=s12[:], scalar=v8[:, j:j + 1], in1=gixf[:],
                        op0=mybir.AluOpType.is_equal, op1=mybir.AluOpType.mult,
                        accum_out=gf8[:, j:j + 1])
                gidx8 = spool.tile([P, 8], u32, tag="gidx8")
                gidx8_t[t] = gidx8
                nc.vector.tensor_copy(out=gidx8[:], in_=gf8[:])

                w8 = spool.tile([P, 8], f32, tag="w8")
                w8_t[t] = w8
                sm8 = spool.tile([P, 1], f32, tag="sm8")
                nc.vector.tensor_scalar(out=w8[:], in0=v8[:], scalar1=v8[:, :1], scalar2=None,
                                        op0=mybir.AluOpType.subtract)
                nc.scalar.activation(out=w8[:], in_=w8[:], func=mybir.ActivationFunctionType.Exp,
                                     accum_out=sm8[:, :1])
                rcp8 = spool.tile([P, 1], f32, tag="rcp8")
                nc.vector.reciprocal(out=rcp8[:], in_=sm8[:, :1])
                nc.vector.tensor_scalar(out=w8[:], in0=w8[:], scalar1=rcp8[:, :1], scalar2=None,
                                        op0=mybir.AluOpType.mult)

                gat = gpool.tile([P, K_TOP * D_MODEL], f32, tag="gat")
                gat_t[t] = gat
                for j in range(K_TOP):
                    nc.gpsimd.indirect_dma_start(
                        out=gat[:, j * D_MODEL:(j + 1) * D_MODEL], out_offset=None, in_=V[:],
                        in_offset=bass.IndirectOffsetOnAxis(ap=gidx8[:, j:j + 1], axis=0))

            def stage_dc(t):
                # ---- Dc: weighted accumulate + store ----
                gat = gat_t[t]
                w8 = w8_t[t]
                acc = apool.tile([P, D_MODEL], f32, tag="acc")
                nc.vector.tensor_scalar(out=acc[:], in0=gat[:, 0:D_MODEL], scalar1=w8[:, 0:1],
                                        scalar2=None, op0=mybir.AluOpType.mult)
                for j in range(1, K_TOP):
                    nc.vector.scalar_tensor_tensor(
                        out=acc[:], in0=gat[:, j * D_MODEL:(j + 1) * D_MODEL], scalar=w8[:, j:j + 1],
                        in1=acc[:], op0=mybir.AluOpType.mult, op1=mybir.AluOpType.add)
                nc.sync.dma_start(out=out[t * P:(t + 1) * P, :], in_=acc[:])

            nc.gpsimd.dma_start(out=kh_sb[:], in_=Kh[:])
            nc.gpsimd.dma_start(out=iota_sb[:], in_=iotaS[:])
            nc.gpsimd.dma_start(out=base_sb[:], in_=baseS[:])

            # software pipeline: tile t+1's B (Wg stream + PE) and tile t's
            # gathers all run during tile t's C Vector work
            stage_ab(0)
            stage_c(0); stage_ab(1); stage_da(0)
            stage_c(1); stage_ab(2); stage_da(1); stage_db(0)
            stage_c(2); stage_ab(3); stage_da(2); stage_dc(0); stage_db(1)
            stage_c(3); stage_da(3); stage_dc(1); stage_db(2)
            stage_dc(2); stage_db(3); stage_dc(3)

    nc.compile()
    return nc


_NC_CACHE = {}


def _get_nc():
    if "v4" not in _NC_CACHE:
        _NC_CACHE["v4"] = _build()
    return _NC_CACHE["v4"]


def _split16(a):
    hi = a.astype(np.float16)
    lo = (a - hi.astype(np.float32)).astype(np.float16)
    return hi, lo


def _prep_in_maps(x, router_w, compress_neurons, knowledge_K, knowledge_V):
    x = np.asarray(x, dtype=np.float32).reshape(B * S, D_MODEL) * XS
    rwT = np.ascontiguousarray(np.asarray(router_w, dtype=np.float32).T) * RWS
    rw_r = np.ascontiguousarray(
        rwT.reshape(N_DC, P, N_COMPRESS).transpose(1, 0, 2).reshape(P, N_DC * N_COMPRESS))
    rwh, rwl = _split16(rw_r)
    cn = np.asarray(compress_neurons, dtype=np.float32) * WS
    Wg1 = np.ascontiguousarray(
        cn.reshape(N_G, 4, N_DC, P, RANK).transpose(0, 2, 3, 1, 4).reshape(N_G * N_DC * P, 4 * RANK))
    Wg = np.ascontiguousarray(np.tile(Wg1, (N_TILES, 1)))   # tile-major replication
    Wgh, Wgl = _split16(Wg)
    K = np.asarray(knowledge_K, dtype=np.float32)
    KT = np.ascontiguousarray(K.T)                           # [128, 32768]
    Kh = KT.astype(np.float16)
    Vf = np.ascontiguousarray(np.asarray(knowledge_V, dtype=np.float32))
    ident = np.eye(P, dtype=np.float32)
    iota = np.tile((np.arange(SUP, dtype=np.float64) * (2.0 ** -26)).astype(np.float32), (P, 1))
    base = np.tile(((np.arange(NCAND) // 8) * SUP).astype(np.float32), (P, 1))
    Krows_f = np.ascontiguousarray(K)

    in_maps = []
    for c in range(N_CORES):
        xs = x[c * TOK_PER_CORE:(c + 1) * TOK_PER_CORE]
        xT = np.ascontiguousarray(
            xs.T.reshape(N_DC, P, TOK_PER_CORE).transpose(1, 0, 2).reshape(P, N_DC * TOK_PER_CORE))
        xhc, xlc = _split16(xT)
        in_maps.append(dict(xh=xhc, xl=xlc, rwh=rwh, rwl=rwl, Wgh=Wgh, Wgl=Wgl,
                            Kh=Kh, Krows=Krows_f, V=Vf, iotaS=iota, baseS=base,
                            ident=ident))
    return in_maps


def _ensure_ntff_hook():
    import sys as _sys
    import types as _types
    if "antenv.axon_hooks" in _sys.modules:
        return
    try:
        import antenv.axon_hooks  # noqa: F401
        return
    except ImportError:
        pass
    mod = _types.ModuleType("antenv.axon_hooks")
    _state = {"hook": None}
    mod.set_axon_ntff_profile_hook = lambda h: _state.__setitem__("hook", h)
    mod.get_axon_ntff_profile_hook = lambda: _state["hook"]
    _sys.modules["antenv.axon_hooks"] = mod
    try:
        from trn_agent_boot.trn_boot import _ntff_profile_via_ctypes
        mod.set_axon_ntff_profile_hook(_ntff_profile_via_ctypes("/opt/axon/libaxon_pjrt.so"))
    except Exception:
        pass


def _run(inputs, trace=False, dbg=False):
    if trace:
        _ensure_ntff_hook()
    nc = _get_nc()
    in_maps = _prep_in_maps(**inputs)
    res = run_bass_kernel_spmd(nc, in_maps, core_ids=list(range(N_CORES)), trace=trace)
    out = np.concatenate([res.results[c]["out"] for c in range(N_CORES)], axis=0)
    return out.reshape(B, S, D_MODEL), res


def kernel(x, router_w, compress_neurons, knowledge_K, knowledge_V):
    out, _ = _run(dict(x=x, router_w=router_w, compress_neurons=compress_neurons,
                       knowledge_K=knowledge_K, knowledge_V=knowledge_V))
    return out


# revision 23
# speedup vs baseline: 1.1263x; 1.0039x over previous
"""NeuronMemory retrieval kernel for 8 TRN2 NeuronCores.

Problem (hardcoded shapes):
  x                [2, 2048, 1024] f32
  router_w         [16, 1024] f32
  compress_neurons [16, 1024, 128] f32
  knowledge_K      [32768, 128] f32
  knowledge_V      [32768, 1024] f32
  out              [2, 2048, 1024] f32

Per token: softmax-routed low-rank projection Q (rank 128), dense scores
against 32768 knowledge keys, top-8, softmax, weighted gather of V rows.

Sharding: data-parallel over the 4096 tokens (512 tokens/core); router,
compress_neurons, knowledge tables replicated on every core. No collectives.

Matmul precision strategy: fp32 matmul on TRN2 runs at 4 cycles/row; fp16
runs at 1. All big matmuls use an fp16 two-term split (a = ah + al,
b = bh + bl, a@b ~= ah@bh + ah@bl + al@bh, error ~2^-22) at 3 cycles/row
equivalent work, with host-side prescaling (x*4, rw*32, W*32, K*32) to
keep fp16 residuals out of the subnormal range. Scale folds:
  router scores' = 128*rs  -> exp scale 1/128 in softmax
  QT' = 16*SCALE*Q         -> ACT copy scale SCALE/8 (Y' = 128*Y)
  knowledge scores' = 512*s -> top-k unchanged (monotone), exp scale 1/512

Per-core pipeline (4 token tiles of 128):
  A. router scores + softmax -> wts [128, 16]
  B. Q' = sum_n wts_n * (x @ W_n) via grouped fp16-split matmuls + fused
     weighting, PE-transpose -> QT' -> fp16 split QTh/QTl
  C. scores' = QT'.T @ K' in 512-col chunks -> PSUM -> SBUF; per
     8192-quarter hardware top-8 (max) + indices (max_index)
  D. merge 32 candidates/token -> exact top-8 + global indices
  E. softmax over top-8 (exp scale 1/512)
  F. indirect-DMA gather of V rows + fused weighted accumulate -> out
"""
import numpy as np

import concourse.bacc as bacc
import concourse.bass as bass
import concourse.mybir as mybir
from concourse.tile import TileContext
from concourse.bass_utils import run_bass_kernel_spmd

P = 128
D_MODEL = 1024
RANK = 128
N_COMPRESS = 16
N_KNOWLEDGE = 32768
K_TOP = 8
B, S = 2, 2048
N_CORES = 8
TOK_PER_CORE = (B * S) // N_CORES      # 512
N_TILES = TOK_PER_CORE // P            # 4
N_DC = D_MODEL // P                    # 8 d-model chunks
N_Q = 4                                # knowledge quarters
QW = N_KNOWLEDGE // N_Q                # 8192 quarter width
N_CH = QW // 512                       # 16 chunks of 512 per quarter
N_G = 4                                # neuron groups of 4
SCALE = 1.0 / np.sqrt(np.float32(RANK))

# host prescales (folded back inside the kernel via ACT scale params)
XS = 4.0
RWS = 32.0
WS = 32.0
KS = 32.0
QT_ACT_SCALE = float(SCALE * 16.0 / 128.0)   # QT' = 16*SCALE*Q from Y' = 128*Y
RT_EXP_SCALE = float(1.0 / (XS * RWS))       # router scores' = 128*rs
SC_EXP_SCALE = float(1.0 / 512.0)            # knowledge scores' = 512*s

f32 = mybir.dt.float32
f16 = mybir.dt.float16
u32 = mybir.dt.uint32


def _build(dbg=False):
    nc = bacc.Bacc("TRN2", target_bir_lowering=False, debug=False, num_devices=N_CORES)

    xh = nc.declare_dram_parameter("xh", [P, N_DC * TOK_PER_CORE], f16, isOutput=False)
    xl = nc.declare_dram_parameter("xl", [P, N_DC * TOK_PER_CORE], f16, isOutput=False)
    rwh = nc.declare_dram_parameter("rwh", [P, N_DC * N_COMPRESS], f16, isOutput=False)
    rwl = nc.declare_dram_parameter("rwl", [P, N_DC * N_COMPRESS], f16, isOutput=False)
    Wgh = nc.declare_dram_parameter("Wgh", [N_G * N_DC * P, 512], f16, isOutput=False)
    Wgl = nc.declare_dram_parameter("Wgl", [N_G * N_DC * P, 512], f16, isOutput=False)
    Kh = nc.declare_dram_parameter("Kh", [P, N_KNOWLEDGE], f16, isOutput=False)
    Kl = nc.declare_dram_parameter("Kl", [P, N_KNOWLEDGE], f16, isOutput=False)
    V0 = nc.declare_dram_parameter("V0", [N_KNOWLEDGE, 512], f32, isOutput=False)
    V1 = nc.declare_dram_parameter("V1", [N_KNOWLEDGE, 512], f32, isOutput=False)
    ident = nc.declare_dram_parameter("ident", [P, P], f32, isOutput=False)
    out = nc.declare_dram_parameter("out", [TOK_PER_CORE, D_MODEL], f32, isOutput=True)

    Wgh_v = Wgh.rearrange("(g dc p) n -> g dc p n", g=N_G, dc=N_DC)
    Wgl_v = Wgl.rearrange("(g dc p) n -> g dc p n", g=N_G, dc=N_DC)

    with TileContext(nc) as tc:
        with (
            tc.tile_pool(name="const", bufs=1) as cpool,
            tc.tile_pool(name="kt", bufs=2) as ktpool,
            tc.tile_pool(name="sc", bufs=2) as scpool,
            tc.tile_pool(name="wld", bufs=3) as wpool,
            tc.tile_pool(name="gat", bufs=2) as gpool,
            tc.tile_pool(name="acc", bufs=2) as apool,
            tc.tile_pool(name="small", bufs=4) as spool,
            tc.tile_pool(name="ps_big", bufs=4, space="PSUM") as psb,
            tc.tile_pool(name="ps_small", bufs=2, space="PSUM") as pss,
        ):
            # ---- persistent loads ----
            xh_sb = cpool.tile([P, N_DC * TOK_PER_CORE], f16)   # 8KB/part
            xl_sb = cpool.tile([P, N_DC * TOK_PER_CORE], f16)
            rwh_sb = cpool.tile([P, N_DC * N_COMPRESS], f16)
            rwl_sb = cpool.tile([P, N_DC * N_COMPRESS], f16)
            id_sb = cpool.tile([P, P], f32)
            nc.sync.dma_start(out=xh_sb[:], in_=xh[:])
            nc.sync.dma_start(out=xl_sb[:], in_=xl[:])
            nc.sync.dma_start(out=rwh_sb[:], in_=rwh[:])
            nc.sync.dma_start(out=rwl_sb[:], in_=rwl[:])
            nc.sync.dma_start(out=id_sb[:], in_=ident[:])

            wts_sb = cpool.tile([P, N_TILES * N_COMPRESS], f32)  # router weights per tile
            Q_sb = cpool.tile([P, N_TILES * RANK], f32)          # [tokens, r] per tile (Q' = 128*Q)
            QT_sb = cpool.tile([P, N_TILES * P], f32)            # QT' = 16*SCALE*Q
            QTh_sb = cpool.tile([P, N_TILES * P], f16)
            QTl_sb = cpool.tile([P, N_TILES * P], f16)
            QThf_sb = cpool.tile([P, N_TILES * P], f32)          # f32 copy of QTh for residual
            cv_sb = cpool.tile([P, N_TILES * N_Q * 8], f32)      # candidate values (512*s)
            cif_sb = cpool.tile([P, N_TILES * N_Q * 8], f32)     # candidate idx as f32

            def tok(t):
                return slice(t * P, (t + 1) * P)

            # ---- A: router softmax (2-term fp16 split, exact to ~2^-22) ----
            for t in range(N_TILES):
                rps = pss.tile([P, N_COMPRESS], f32, space="PSUM", tag="rps")
                n_mm = N_DC * 3
                i_mm = 0
                for dc in range(N_DC):
                    xsl = slice(dc * TOK_PER_CORE + t * P, dc * TOK_PER_CORE + (t + 1) * P)
                    rsl = slice(dc * N_COMPRESS, (dc + 1) * N_COMPRESS)
                    for lhs, rhs in ((xh_sb, rwh_sb), (xh_sb, rwl_sb), (xl_sb, rwh_sb)):
                        nc.tensor.matmul(
                            out=rps[:], lhsT=lhs[:, xsl], rhs=rhs[:, rsl],
                            start=(i_mm == 0), stop=(i_mm == n_mm - 1),
                        )
                        i_mm += 1
                w = wts_sb[:, t * N_COMPRESS:(t + 1) * N_COMPRESS]
                mx = spool.tile([P, 1], f32, tag="mx")
                sm = spool.tile([P, 1], f32, tag="sm")
                ex = spool.tile([P, N_COMPRESS], f32, tag="ex")
                nc.vector.tensor_reduce(out=mx[:], in_=rps[:], op=mybir.AluOpType.max, axis=mybir.AxisListType.X)
                nc.vector.tensor_scalar(out=ex[:], in0=rps[:], scalar1=mx[:, :1], scalar2=None, op0=mybir.AluOpType.subtract)
                nc.scalar.activation(out=ex[:], in_=ex[:], func=mybir.ActivationFunctionType.Exp,
                                     scale=RT_EXP_SCALE, accum_out=sm[:, :1])
                rcp = spool.tile([P, 1], f32, tag="rcp")
                nc.vector.reciprocal(out=rcp[:], in_=sm[:, :1])
                nc.vector.tensor_scalar(out=w, in0=ex[:], scalar1=rcp[:, :1], scalar2=None, op0=mybir.AluOpType.mult)

            # ---- B: Q projection (fp16 3-term split) ----
            yps_tiles = {}
            for g in range(N_G):
                for dc in range(N_DC):
                    wh = wpool.tile([P, 512], f16, tag="wldh")
                    wl = wpool.tile([P, 512], f16, tag="wldl")
                    nc.sync.dma_start(out=wh[:], in_=Wgh_v[g, dc])
                    nc.sync.dma_start(out=wl[:], in_=Wgl_v[g, dc])
                    for t in range(N_TILES):
                        if dc == 0:
                            yps_tiles[t] = psb.tile([P, 512], f32, space="PSUM", tag="ps", name=f"yps_{g}_{t}")
                        xsl = slice(dc * TOK_PER_CORE + t * P, dc * TOK_PER_CORE + (t + 1) * P)
                        for j, (lhs, rhs) in enumerate(((xh_sb, wh), (xh_sb, wl), (xl_sb, wh))):
                            nc.tensor.matmul(
                                out=yps_tiles[t][:], lhsT=lhs[:, xsl], rhs=rhs[:],
                                start=(dc == 0 and j == 0), stop=(dc == N_DC - 1 and j == 2),
                            )
                for t in range(N_TILES):
                    q = Q_sb[:, t * RANK:(t + 1) * RANK]
                    for n in range(4):
                        ncomp = g * 4 + n
                        wcol = wts_sb[:, t * N_COMPRESS + ncomp:t * N_COMPRESS + ncomp + 1]
                        ypart = yps_tiles[t][:, n * RANK:(n + 1) * RANK]
                        if g == 0 and n == 0:
                            nc.vector.tensor_scalar(out=q, in0=ypart, scalar1=wcol, scalar2=None,
                                                    op0=mybir.AluOpType.mult)
                        else:
                            nc.vector.scalar_tensor_tensor(out=q, in0=ypart, scalar=wcol, in1=q,
                                                           op0=mybir.AluOpType.mult,
                                                           op1=mybir.AluOpType.add)

            # transpose Q' -> QT', scale to QT' = 16*SCALE*Q, then fp16 split
            for t in range(N_TILES):
                tps = pss.tile([P, P], f32, space="PSUM", tag="tps")
                nc.tensor.transpose(out=tps[:], in_=Q_sb[:, t * RANK:(t + 1) * RANK], identity=id_sb[:])
                nc.scalar.activation(out=QT_sb[:, tok(t)], in_=tps[:],
                                     func=mybir.ActivationFunctionType.Copy, scale=QT_ACT_SCALE)
                nc.vector.tensor_copy(out=QTh_sb[:, tok(t)], in_=QT_sb[:, tok(t)])
                nc.vector.tensor_copy(out=QThf_sb[:, tok(t)], in_=QTh_sb[:, tok(t)])
                nc.vector.tensor_tensor(out=QTl_sb[:, tok(t)], in0=QT_sb[:, tok(t)],
                                        in1=QThf_sb[:, tok(t)], op=mybir.AluOpType.subtract)

            # ---- C: knowledge scores + per-quarter top8 ----
            for q in range(N_Q):
                kh = ktpool.tile([P, QW], f16, tag="ktqh")
                kl = ktpool.tile([P, QW], f16, tag="ktql")
                nc.sync.dma_start(out=kh[:], in_=Kh[:, q * QW:(q + 1) * QW])
                nc.sync.dma_start(out=kl[:], in_=Kl[:, q * QW:(q + 1) * QW])
                for t in range(N_TILES):
                    sc = scpool.tile([P, QW], f32, tag="sc")
                    for c in range(N_CH):
                        sps = psb.tile([P, 512], f32, space="PSUM", tag="ps")
                        ksl = slice(c * 512, (c + 1) * 512)
                        for j, (lhs, rhs) in enumerate(
                                ((QTh_sb, kh), (QTh_sb, kl), (QTl_sb, kh))):
                            nc.tensor.matmul(
                                out=sps[:], lhsT=lhs[:, tok(t)], rhs=rhs[:, ksl],
                                start=(j == 0), stop=(j == 2),
                            )
                        nc.scalar.copy(out=sc[:, c * 512:(c + 1) * 512], in_=sps[:])
                    vq = cv_sb[:, (t * N_Q + q) * 8:(t * N_Q + q + 1) * 8]
                    iq = spool.tile([P, 8], u32, tag="iq")
                    nc.vector.max(out=vq, in_=sc[:])
                    nc.vector.max_index(out=iq[:], in_max=vq, in_values=sc[:])
                    nc.vector.tensor_copy(out=cif_sb[:, (t * N_Q + q) * 8:(t * N_Q + q + 1) * 8], in_=iq[:])
                    if q > 0:
                        nc.vector.tensor_scalar(
                            out=cif_sb[:, (t * N_Q + q) * 8:(t * N_Q + q + 1) * 8],
                            in0=cif_sb[:, (t * N_Q + q) * 8:(t * N_Q + q + 1) * 8],
                            scalar1=float(q * QW), scalar2=None, op0=mybir.AluOpType.add)

            # ---- D/E/F per tile ----
            NCAND = N_Q * 8
            for t in range(N_TILES):
                cv = cv_sb[:, t * NCAND:(t + 1) * NCAND]
                cif = cif_sb[:, t * NCAND:(t + 1) * NCAND]
                v8 = spool.tile([P, 8], f32, tag="v8")
                nc.vector.max(out=v8[:], in_=cv)
                idxf = spool.tile([P, 8], f32, tag="idxf")
                junk = spool.tile([P, NCAND], f32, tag="junk")
                for j in range(K_TOP):
                    nc.vector.scalar_tensor_tensor(
                        out=junk[:], in0=cv, scalar=v8[:, j:j + 1], in1=cif,
                        op0=mybir.AluOpType.is_equal, op1=mybir.AluOpType.mult,
                        accum_out=idxf[:, j:j + 1])
                gidx = spool.tile([P, 8], u32, tag="gidx")
                nc.vector.tensor_copy(out=gidx[:], in_=idxf[:])

                # softmax over top8 of scores' = 512*s -> exp scale 1/512
                w8 = spool.tile([P, 8], f32, tag="w8")
                sm8 = spool.tile([P, 1], f32, tag="sm8")
                nc.vector.tensor_scalar(out=w8[:], in0=v8[:], scalar1=v8[:, :1], scalar2=None,
                                        op0=mybir.AluOpType.subtract)
                nc.scalar.activation(out=w8[:], in_=w8[:], func=mybir.ActivationFunctionType.Exp,
                                     scale=SC_EXP_SCALE, accum_out=sm8[:, :1])
                rcp8 = spool.tile([P, 1], f32, tag="rcp8")
                nc.vector.reciprocal(out=rcp8[:], in_=sm8[:, :1])
                nc.vector.tensor_scalar(out=w8[:], in0=w8[:], scalar1=rcp8[:, :1], scalar2=None,
                                        op0=mybir.AluOpType.mult)

                # gather V rows (two 512-wide halves) + weighted accumulate
                for dh, Vh in ((0, V0), (1, V1)):
                    gat = gpool.tile([P, K_TOP * 512], f32, tag="gat")
                    for j in range(K_TOP):
                        nc.gpsimd.indirect_dma_start(
                            out=gat[:, j * 512:(j + 1) * 512],
                            out_offset=None,
                            in_=Vh[:],
                            in_offset=bass.IndirectOffsetOnAxis(ap=gidx[:, j:j + 1], axis=0),
                        )
                    acc = apool.tile([P, 512], f32, tag="acc")
                    nc.vector.tensor_scalar(out=acc[:], in0=gat[:, 0:512], scalar1=w8[:, 0:1],
                                            scalar2=None, op0=mybir.AluOpType.mult)
                    for j in range(1, K_TOP):
                        nc.vector.scalar_tensor_tensor(
                            out=acc[:], in0=gat[:, j * 512:(j + 1) * 512], scalar=w8[:, j:j + 1],
                            in1=acc[:], op0=mybir.AluOpType.mult, op1=mybir.AluOpType.add)
                    nc.sync.dma_start(out=out[t * P:(t + 1) * P, dh * 512:(dh + 1) * 512], in_=acc[:])

    nc.compile()
    return nc


_NC_CACHE = {}


def _get_nc(dbg=False):
    if dbg not in _NC_CACHE:
        _NC_CACHE[dbg] = _build(dbg)
    return _NC_CACHE[dbg]


def _split16(a):
    hi = a.astype(np.float16)
    lo = (a - hi.astype(np.float32)).astype(np.float16)
    return hi, lo


def _prep_in_maps(x, router_w, compress_neurons, knowledge_K, knowledge_V):
    x = np.asarray(x, dtype=np.float32).reshape(B * S, D_MODEL) * XS
    rwT = np.ascontiguousarray(np.asarray(router_w, dtype=np.float32).T) * RWS  # [1024, 16]
    rw_r = np.ascontiguousarray(
        rwT.reshape(N_DC, P, N_COMPRESS).transpose(1, 0, 2).reshape(P, N_DC * N_COMPRESS))
    rwh, rwl = _split16(rw_r)
    cn = np.asarray(compress_neurons, dtype=np.float32) * WS
    Wg = np.ascontiguousarray(
        cn.reshape(N_G, 4, N_DC, P, RANK).transpose(0, 2, 3, 1, 4).reshape(N_G * N_DC * P, 4 * RANK))
    Wgh, Wgl = _split16(Wg)
    KT = np.ascontiguousarray(np.asarray(knowledge_K, dtype=np.float32).T) * KS  # [128, 32768]
    Kh, Kl = _split16(KT)
    V = np.asarray(knowledge_V, dtype=np.float32)
    V0 = np.ascontiguousarray(V[:, :512])
    V1 = np.ascontiguousarray(V[:, 512:])
    ident = np.eye(P, dtype=np.float32)

    in_maps = []
    for c in range(N_CORES):
        xs = x[c * TOK_PER_CORE:(c + 1) * TOK_PER_CORE]                        # [512, 1024]
        xT = np.ascontiguousarray(
            xs.T.reshape(N_DC, P, TOK_PER_CORE).transpose(1, 0, 2).reshape(P, N_DC * TOK_PER_CORE))
        xhc, xlc = _split16(xT)
        in_maps.append(dict(xh=xhc, xl=xlc, rwh=rwh, rwl=rwl, Wgh=Wgh, Wgl=Wgl,
                            Kh=Kh, Kl=Kl, V0=V0, V1=V1, ident=ident))
    return in_maps


def _ensure_ntff_hook():
    import sys as _sys
    import types as _types
    if "antenv.axon_hooks" in _sys.modules:
        return
    try:
        import antenv.axon_hooks  # noqa: F401
        return
    except ImportError:
        pass
    mod = _types.ModuleType("antenv.axon_hooks")
    _state = {"hook": None}
    mod.set_axon_ntff_profile_hook = lambda h: _state.__setitem__("hook", h)
    mod.get_axon_ntff_profile_hook = lambda: _state["hook"]
    _sys.modules["antenv.axon_hooks"] = mod
    try:
        from trn_agent_boot.trn_boot import _ntff_profile_via_ctypes
        mod.set_axon_ntff_profile_hook(_ntff_profile_via_ctypes("/opt/axon/libaxon_pjrt.so"))
    except Exception:
        pass


def _run(inputs, trace=False, dbg=False):
    if trace:
        _ensure_ntff_hook()
    nc = _get_nc(dbg)
    in_maps = _prep_in_maps(**inputs)
    res = run_bass_kernel_spmd(nc, in_maps, core_ids=list(range(N_CORES)), trace=trace)
    out = np.concatenate([res.results[c]["out"] for c in range(N_CORES)], axis=0)
    return out.reshape(B, S, D_MODEL), res


def kernel(x, router_w, compress_neurons, knowledge_K, knowledge_V):
    out, _ = _run(dict(x=x, router_w=router_w, compress_neurons=compress_neurons,
                       knowledge_K=knowledge_K, knowledge_V=knowledge_V))
    return out


# revision 24
# speedup vs baseline: 1.1754x; 1.0437x over previous
"""NeuronMemory retrieval kernel for 8 TRN2 NeuronCores.

Per token: softmax-routed low-rank projection Q (rank 128), dense scores
against 32768 knowledge keys, top-8, softmax, weighted gather of V rows.
Sharding: data-parallel over the 4096 tokens (512/core); tables replicated.

Matmul precision: fp16 two-term splits (a = ah+al, b = bh+bl,
a@b ~= ah@bh + ah@bl + al@bh, err ~2^-22) with host prescaling (x*4,
rw*32, W*32, K*32) folded back via activation scale params. Selection is
bit-faithful to the fp32 reference (0 mismatched tokens).

Schedule: tiles processed in PAIRS. Pair 1's B-phase (Wg weight stream +
PE matmuls) is issued right after pair 0's C-phase matmuls, so it runs
while the Vector engine does pair 0's top-k passes. The gather/accumulate
tail is software-pipelined in 3 stages (merge+gather-h0 / accum-h0+
gather-h1 / accum-h1) so indirect-DMA latency hides under Vector work.
"""
import numpy as np

import concourse.bacc as bacc
import concourse.bass as bass
import concourse.mybir as mybir
from concourse.tile import TileContext
from concourse.bass_utils import run_bass_kernel_spmd

P = 128
D_MODEL = 1024
RANK = 128
N_COMPRESS = 16
N_KNOWLEDGE = 32768
K_TOP = 8
B, S = 2, 2048
N_CORES = 8
TOK_PER_CORE = (B * S) // N_CORES      # 512
N_TILES = TOK_PER_CORE // P            # 4
N_DC = D_MODEL // P                    # 8
N_Q = 4                                # knowledge quarters
QW = N_KNOWLEDGE // N_Q                # 8192
N_CH = QW // 512                       # 16 chunks of 512 per quarter
N_G = 4                                # neuron groups of 4
SCALE = 1.0 / np.sqrt(np.float32(RANK))

XS = 4.0
RWS = 32.0
WS = 32.0
KS = 32.0
QT_ACT_SCALE = float(SCALE * 16.0 / 128.0)   # QT' = 16*SCALE*Q from Y' = 128*Y
RT_EXP_SCALE = float(1.0 / (XS * RWS))       # router scores' = 128*rs
SC_EXP_SCALE = float(1.0 / 512.0)            # knowledge scores' = 512*s

f32 = mybir.dt.float32
f16 = mybir.dt.float16
u32 = mybir.dt.uint32


def _build():
    nc = bacc.Bacc("TRN2", target_bir_lowering=False, debug=False, num_devices=N_CORES)

    xh = nc.declare_dram_parameter("xh", [P, N_DC * TOK_PER_CORE], f16, isOutput=False)
    xl = nc.declare_dram_parameter("xl", [P, N_DC * TOK_PER_CORE], f16, isOutput=False)
    rwh = nc.declare_dram_parameter("rwh", [P, N_DC * N_COMPRESS], f16, isOutput=False)
    rwl = nc.declare_dram_parameter("rwl", [P, N_DC * N_COMPRESS], f16, isOutput=False)
    Wgh = nc.declare_dram_parameter("Wgh", [N_G * N_DC * P, 512], f16, isOutput=False)
    Wgl = nc.declare_dram_parameter("Wgl", [N_G * N_DC * P, 512], f16, isOutput=False)
    Kh = nc.declare_dram_parameter("Kh", [P, N_KNOWLEDGE], f16, isOutput=False)
    Kl = nc.declare_dram_parameter("Kl", [P, N_KNOWLEDGE], f16, isOutput=False)
    V0 = nc.declare_dram_parameter("V0", [N_KNOWLEDGE, 512], f32, isOutput=False)
    V1 = nc.declare_dram_parameter("V1", [N_KNOWLEDGE, 512], f32, isOutput=False)
    ident = nc.declare_dram_parameter("ident", [P, P], f32, isOutput=False)
    out = nc.declare_dram_parameter("out", [TOK_PER_CORE, D_MODEL], f32, isOutput=True)

    Wgh_v = Wgh.rearrange("(g dc p) n -> g dc p n", g=N_G, dc=N_DC)
    Wgl_v = Wgl.rearrange("(g dc p) n -> g dc p n", g=N_G, dc=N_DC)

    with TileContext(nc) as tc:
        with (
            tc.tile_pool(name="const", bufs=1) as cpool,
            tc.tile_pool(name="kt", bufs=2) as ktpool,
            tc.tile_pool(name="sc", bufs=2) as scpool,
            tc.tile_pool(name="wld", bufs=4) as wpool,
            tc.tile_pool(name="gat", bufs=2) as gpool,
            tc.tile_pool(name="acc", bufs=2) as apool,
            tc.tile_pool(name="small", bufs=6) as spool,
            tc.tile_pool(name="ps_big", bufs=4, space="PSUM") as psb,
            tc.tile_pool(name="ps_small", bufs=2, space="PSUM") as pss,
        ):
            # ---- persistent loads ----
            xh_sb = cpool.tile([P, N_DC * TOK_PER_CORE], f16)
            xl_sb = cpool.tile([P, N_DC * TOK_PER_CORE], f16)
            rwh_sb = cpool.tile([P, N_DC * N_COMPRESS], f16)
            rwl_sb = cpool.tile([P, N_DC * N_COMPRESS], f16)
            id_sb = cpool.tile([P, P], f32)
            nc.sync.dma_start(out=xh_sb[:], in_=xh[:])
            nc.sync.dma_start(out=xl_sb[:], in_=xl[:])
            nc.sync.dma_start(out=rwh_sb[:], in_=rwh[:])
            nc.sync.dma_start(out=rwl_sb[:], in_=rwl[:])
            nc.sync.dma_start(out=id_sb[:], in_=ident[:])

            wts_sb = cpool.tile([P, N_TILES * N_COMPRESS], f32)
            QT_sb = cpool.tile([P, N_TILES * P], f32)
            QTh_sb = cpool.tile([P, N_TILES * P], f16)
            QTl_sb = cpool.tile([P, N_TILES * P], f16)
            QThf_sb = cpool.tile([P, N_TILES * P], f32)
            Q_sb = cpool.tile([P, N_TILES * RANK], f32)
            cv_sb = cpool.tile([P, N_TILES * N_Q * 8], f32)
            cif_sb = cpool.tile([P, N_TILES * N_Q * 8], f32)

            def tok(t):
                return slice(t * P, (t + 1) * P)

            def stage_a(t):
                rps = pss.tile([P, N_COMPRESS], f32, space="PSUM", tag="rps")
                n_mm = N_DC * 3
                i_mm = 0
                for dc in range(N_DC):
                    xsl = slice(dc * TOK_PER_CORE + t * P, dc * TOK_PER_CORE + (t + 1) * P)
                    rsl = slice(dc * N_COMPRESS, (dc + 1) * N_COMPRESS)
                    for lhs, rhs in ((xh_sb, rwh_sb), (xh_sb, rwl_sb), (xl_sb, rwh_sb)):
                        nc.tensor.matmul(out=rps[:], lhsT=lhs[:, xsl], rhs=rhs[:, rsl],
                                         start=(i_mm == 0), stop=(i_mm == n_mm - 1))
                        i_mm += 1
                w = wts_sb[:, t * N_COMPRESS:(t + 1) * N_COMPRESS]
                mx = spool.tile([P, 1], f32, tag="mx")
                sm = spool.tile([P, 1], f32, tag="sm")
                ex = spool.tile([P, N_COMPRESS], f32, tag="ex")
                nc.vector.tensor_reduce(out=mx[:], in_=rps[:], op=mybir.AluOpType.max, axis=mybir.AxisListType.X)
                nc.vector.tensor_scalar(out=ex[:], in0=rps[:], scalar1=mx[:, :1], scalar2=None, op0=mybir.AluOpType.subtract)
                nc.scalar.activation(out=ex[:], in_=ex[:], func=mybir.ActivationFunctionType.Exp,
                                     scale=RT_EXP_SCALE, accum_out=sm[:, :1])
                rcp = spool.tile([P, 1], f32, tag="rcp")
                nc.vector.reciprocal(out=rcp[:], in_=sm[:, :1])
                nc.vector.tensor_scalar(out=w, in0=ex[:], scalar1=rcp[:, :1], scalar2=None, op0=mybir.AluOpType.mult)

            def stage_bt(tiles):
                # B: Q' for this pair of tiles (fp16 3-term); Wg streamed once
                yps_tiles = {}
                for g in range(N_G):
                    for dc in range(N_DC):
                        wh = wpool.tile([P, 512], f16, tag="wldh")
                        wl = wpool.tile([P, 512], f16, tag="wldl")
                        nc.sync.dma_start(out=wh[:], in_=Wgh_v[g, dc])
                        nc.sync.dma_start(out=wl[:], in_=Wgl_v[g, dc])
                        for t in tiles:
                            if dc == 0:
                                yps_tiles[t] = psb.tile([P, 512], f32, space="PSUM", tag="ps",
                                                        name=f"yps_{g}_{t}")
                            xsl = slice(dc * TOK_PER_CORE + t * P, dc * TOK_PER_CORE + (t + 1) * P)
                            for j, (lhs, rhs) in enumerate(((xh_sb, wh), (xh_sb, wl), (xl_sb, wh))):
                                nc.tensor.matmul(out=yps_tiles[t][:], lhsT=lhs[:, xsl], rhs=rhs[:],
                                                 start=(dc == 0 and j == 0),
                                                 stop=(dc == N_DC - 1 and j == 2))
                    for t in tiles:
                        q = Q_sb[:, t * RANK:(t + 1) * RANK]
                        for n in range(4):
                            ncomp = g * 4 + n
                            wcol = wts_sb[:, t * N_COMPRESS + ncomp:t * N_COMPRESS + ncomp + 1]
                            ypart = yps_tiles[t][:, n * RANK:(n + 1) * RANK]
                            if g == 0 and n == 0:
                                nc.vector.tensor_scalar(out=q, in0=ypart, scalar1=wcol, scalar2=None,
                                                        op0=mybir.AluOpType.mult)
                            else:
                                nc.vector.scalar_tensor_tensor(out=q, in0=ypart, scalar=wcol, in1=q,
                                                               op0=mybir.AluOpType.mult,
                                                               op1=mybir.AluOpType.add)
                # T: transpose -> QT' -> fp16 split
                for t in tiles:
                    tps = pss.tile([P, P], f32, space="PSUM", tag="tps")
                    nc.tensor.transpose(out=tps[:], in_=Q_sb[:, t * RANK:(t + 1) * RANK], identity=id_sb[:])
                    nc.scalar.activation(out=QT_sb[:, tok(t)], in_=tps[:],
                                         func=mybir.ActivationFunctionType.Copy, scale=QT_ACT_SCALE)
                    nc.vector.tensor_copy(out=QTh_sb[:, tok(t)], in_=QT_sb[:, tok(t)])
                    nc.vector.tensor_copy(out=QThf_sb[:, tok(t)], in_=QTh_sb[:, tok(t)])
                    nc.vector.tensor_tensor(out=QTl_sb[:, tok(t)], in0=QT_sb[:, tok(t)],
                                            in1=QThf_sb[:, tok(t)], op=mybir.AluOpType.subtract)

            def stage_c(tiles):
                # C: knowledge scores + per-quarter top8 for this pair
                for q in range(N_Q):
                    kh = ktpool.tile([P, QW], f16, tag="ktqh")
                    kl = ktpool.tile([P, QW], f16, tag="ktql")
                    nc.sync.dma_start(out=kh[:], in_=Kh[:, q * QW:(q + 1) * QW])
                    nc.sync.dma_start(out=kl[:], in_=Kl[:, q * QW:(q + 1) * QW])
                    for t in tiles:
                        sc = scpool.tile([P, QW], f32, tag="sc")
                        for c in range(N_CH):
                            sps = psb.tile([P, 512], f32, space="PSUM", tag="ps")
                            ksl = slice(c * 512, (c + 1) * 512)
                            for j, (lhs, rhs) in enumerate(
                                    ((QTh_sb, kh), (QTh_sb, kl), (QTl_sb, kh))):
                                nc.tensor.matmul(out=sps[:], lhsT=lhs[:, tok(t)], rhs=rhs[:, ksl],
                                                 start=(j == 0), stop=(j == 2))
                            nc.scalar.copy(out=sc[:, c * 512:(c + 1) * 512], in_=sps[:])
                        vq = cv_sb[:, (t * N_Q + q) * 8:(t * N_Q + q + 1) * 8]
                        iq = spool.tile([P, 8], u32, tag="iq")
                        nc.vector.max(out=vq, in_=sc[:])
                        nc.vector.max_index(out=iq[:], in_max=vq, in_values=sc[:])
                        nc.vector.tensor_copy(out=cif_sb[:, (t * N_Q + q) * 8:(t * N_Q + q + 1) * 8], in_=iq[:])
                        if q > 0:
                            nc.vector.tensor_scalar(
                                out=cif_sb[:, (t * N_Q + q) * 8:(t * N_Q + q + 1) * 8],
                                in0=cif_sb[:, (t * N_Q + q) * 8:(t * N_Q + q + 1) * 8],
                                scalar1=float(q * QW), scalar2=None, op0=mybir.AluOpType.add)

            NCAND = N_Q * 8
            w8_t = {}
            gidx_t = {}
            gat_t = {}

            def stage_da(t):
                # merge candidates -> exact top8 + global indices + softmax,
                # then issue the half-0 V gather
                cv = cv_sb[:, t * NCAND:(t + 1) * NCAND]
                cif = cif_sb[:, t * NCAND:(t + 1) * NCAND]
                v8 = spool.tile([P, 8], f32, tag="v8")
                nc.vector.max(out=v8[:], in_=cv)
                idxf = spool.tile([P, 8], f32, tag="idxf")
                junk = spool.tile([P, NCAND], f32, tag="junk")
                for j in range(K_TOP):
                    nc.vector.scalar_tensor_tensor(
                        out=junk[:], in0=cv, scalar=v8[:, j:j + 1], in1=cif,
                        op0=mybir.AluOpType.is_equal, op1=mybir.AluOpType.mult,
                        accum_out=idxf[:, j:j + 1])
                gidx = spool.tile([P, 8], u32, tag="gidx")
                gidx_t[t] = gidx
                nc.vector.tensor_copy(out=gidx[:], in_=idxf[:])

                w8 = spool.tile([P, 8], f32, tag="w8")
                w8_t[t] = w8
                sm8 = spool.tile([P, 1], f32, tag="sm8")
                nc.vector.tensor_scalar(out=w8[:], in0=v8[:], scalar1=v8[:, :1], scalar2=None,
                                        op0=mybir.AluOpType.subtract)
                nc.scalar.activation(out=w8[:], in_=w8[:], func=mybir.ActivationFunctionType.Exp,
                                     scale=SC_EXP_SCALE, accum_out=sm8[:, :1])
                rcp8 = spool.tile([P, 1], f32, tag="rcp8")
                nc.vector.reciprocal(out=rcp8[:], in_=sm8[:, :1])
                nc.vector.tensor_scalar(out=w8[:], in0=w8[:], scalar1=rcp8[:, :1], scalar2=None,
                                        op0=mybir.AluOpType.mult)

                gat = gpool.tile([P, K_TOP * 512], f32, tag="gat")
                gat_t[t] = gat
                for j in range(K_TOP):
                    nc.gpsimd.indirect_dma_start(
                        out=gat[:, j * 512:(j + 1) * 512], out_offset=None, in_=V0[:],
                        in_offset=bass.IndirectOffsetOnAxis(ap=gidx[:, j:j + 1], axis=0))

            def stage_db(t):
                # accumulate half 0, store, then issue the half-1 gather
                gat = gat_t[t]
                w8 = w8_t[t]
                acc = apool.tile([P, 512], f32, tag="acc")
                nc.vector.tensor_scalar(out=acc[:], in0=gat[:, 0:512], scalar1=w8[:, 0:1],
                                        scalar2=None, op0=mybir.AluOpType.mult)
                for j in range(1, K_TOP):
                    nc.vector.scalar_tensor_tensor(
                        out=acc[:], in0=gat[:, j * 512:(j + 1) * 512], scalar=w8[:, j:j + 1],
                        in1=acc[:], op0=mybir.AluOpType.mult, op1=mybir.AluOpType.add)
                nc.sync.dma_start(out=out[t * P:(t + 1) * P, 0:512], in_=acc[:])
                gat2 = gpool.tile([P, K_TOP * 512], f32, tag="gat")
                gat_t[t] = gat2
                for j in range(K_TOP):
                    nc.gpsimd.indirect_dma_start(
                        out=gat2[:, j * 512:(j + 1) * 512], out_offset=None, in_=V1[:],
                        in_offset=bass.IndirectOffsetOnAxis(ap=gidx_t[t][:, j:j + 1], axis=0))

            def stage_dc(t):
                # accumulate half 1 and store
                gat = gat_t[t]
                w8 = w8_t[t]
                acc = apool.tile([P, 512], f32, tag="acc")
                nc.vector.tensor_scalar(out=acc[:], in0=gat[:, 0:512], scalar1=w8[:, 0:1],
                                        scalar2=None, op0=mybir.AluOpType.mult)
                for j in range(1, K_TOP):
                    nc.vector.scalar_tensor_tensor(
                        out=acc[:], in0=gat[:, j * 512:(j + 1) * 512], scalar=w8[:, j:j + 1],
                        in1=acc[:], op0=mybir.AluOpType.mult, op1=mybir.AluOpType.add)
                nc.sync.dma_start(out=out[t * P:(t + 1) * P, 512:1024], in_=acc[:])

            # ---- paired schedule ----
            stage_a(0); stage_a(1)
            stage_bt((0, 1))
            stage_c((0, 1))
            stage_da(0); stage_da(1)
            stage_a(2); stage_a(3)
            stage_bt((2, 3))
            stage_c((2, 3))
            stage_db(0); stage_db(1)
            stage_dc(0); stage_dc(1)
            stage_da(2); stage_da(3)
            stage_db(2); stage_db(3)
            stage_dc(2); stage_dc(3)

    nc.compile()
    return nc


_NC_CACHE = {}


def _get_nc():
    if "k" not in _NC_CACHE:
        _NC_CACHE["k"] = _build()
    return _NC_CACHE["k"]


def _split16(a):
    hi = a.astype(np.float16)
    lo = (a - hi.astype(np.float32)).astype(np.float16)
    return hi, lo


def _prep_in_maps(x, router_w, compress_neurons, knowledge_K, knowledge_V):
    x = np.asarray(x, dtype=np.float32).reshape(B * S, D_MODEL) * XS
    rwT = np.ascontiguousarray(np.asarray(router_w, dtype=np.float32).T) * RWS
    rw_r = np.ascontiguousarray(
        rwT.reshape(N_DC, P, N_COMPRESS).transpose(1, 0, 2).reshape(P, N_DC * N_COMPRESS))
    rwh, rwl = _split16(rw_r)
    cn = np.asarray(compress_neurons, dtype=np.float32) * WS
    Wg = np.ascontiguousarray(
        cn.reshape(N_G, 4, N_DC, P, RANK).transpose(0, 2, 3, 1, 4).reshape(N_G * N_DC * P, 4 * RANK))
    Wgh, Wgl = _split16(Wg)
    KT = np.ascontiguousarray(np.asarray(knowledge_K, dtype=np.float32).T) * KS
    Kh, Kl = _split16(KT)
    V = np.asarray(knowledge_V, dtype=np.float32)
    V0 = np.ascontiguousarray(V[:, :512])
    V1 = np.ascontiguousarray(V[:, 512:])
    ident = np.eye(P, dtype=np.float32)

    in_maps = []
    for c in range(N_CORES):
        xs = x[c * TOK_PER_CORE:(c + 1) * TOK_PER_CORE]
        xT = np.ascontiguousarray(
            xs.T.reshape(N_DC, P, TOK_PER_CORE).transpose(1, 0, 2).reshape(P, N_DC * TOK_PER_CORE))
        xhc, xlc = _split16(xT)
        in_maps.append(dict(xh=xhc, xl=xlc, rwh=rwh, rwl=rwl, Wgh=Wgh, Wgl=Wgl,
                            Kh=Kh, Kl=Kl, V0=V0, V1=V1, ident=ident))
    return in_maps


def _ensure_ntff_hook():
    import sys as _sys
    import types as _types
    if "antenv.axon_hooks" in _sys.modules:
        return
    try:
        import antenv.axon_hooks  # noqa: F401
        return
    except ImportError:
        pass
    mod = _types.ModuleType("antenv.axon_hooks")
    _state = {"hook": None}
    mod.set_axon_ntff_profile_hook = lambda h: _state.__setitem__("hook", h)
    mod.get_axon_ntff_profile_hook = lambda: _state["hook"]
    _sys.modules["antenv.axon_hooks"] = mod
    try:
        from trn_agent_boot.trn_boot import _ntff_profile_via_ctypes
        mod.set_axon_ntff_profile_hook(_ntff_profile_via_ctypes("/opt/axon/libaxon_pjrt.so"))
    except Exception:
        pass


def _run(inputs, trace=False, dbg=False):
    if trace:
        _ensure_ntff_hook()
    nc = _get_nc()
    in_maps = _prep_in_maps(**inputs)
    res = run_bass_kernel_spmd(nc, in_maps, core_ids=list(range(N_CORES)), trace=trace)
    out = np.concatenate([res.results[c]["out"] for c in range(N_CORES)], axis=0)
    return out.reshape(B, S, D_MODEL), res


def kernel(x, router_w, compress_neurons, knowledge_K, knowledge_V):
    out, _ = _run(dict(x=x, router_w=router_w, compress_neurons=compress_neurons,
                       knowledge_K=knowledge_K, knowledge_V=knowledge_V))
    return out
